# revision 8
# baseline (speedup 1.0000x reference)
"""Fused causal GQA attention block (RMSNorm+RoPE+value-residual+gated attn)
for Trainium2, SPMD over 8 NeuronCores.

Sharding: core c = b*4 + g  (b in {0,1} batch, g in {0..3} kv-head group).
Each core computes its batch's 4 q-heads (one kv head) end-to-end:
  Q/K/V projections, RMSNorm+RoPE, causal softmax (no max-sub needed:
  |scores| <= sqrt(128)), value-residual lerp (folded into Wv/v1 on host),
  sigmoid gating, and a partial output projection (its 512 rows of Wo).
Host sums the 4 partial out-projections per batch.

x is shipped pre-transposed (X^T) so no on-device transposition of the
activations is needed; all matmuls run as float32r (e8m11) on the PE at
bf16 speed. Softmax denominators are partition-reduced on the (otherwise
idle) GPSIMD engine instead of PE ones-matmuls.
"""

import os
import sys
import numpy as np

for _p in ("/opt/trn_rl_repo", "/root/.axon_site/_ro/trn_rl_repo"):
    if os.path.isdir(_p) and _p not in sys.path:
        sys.path.insert(0, _p)

import concourse.bass as bass  # noqa: E402
import concourse.mybir as mybir  # noqa: E402
import concourse.tile as tile  # noqa: E402
from concourse import bacc  # noqa: E402
from concourse.bass_utils import run_bass_kernel_spmd  # noqa: E402
from concourse.masks import make_identity  # noqa: E402

AF = mybir.ActivationFunctionType
F32 = mybir.dt.float32
F32R = mybir.dt.float32r

B, T, D = 2, 2048, 2048
NH, NKV, DH = 16, 4, 128
HLOC = NH // NKV          # 4 q heads per core
P = 128                   # partition tile
NT = T // P               # 16 token tiles
NKC = D // P              # 16 contraction chunks
NTB = 4                   # tq blocks of 512
TB = T // NTB             # 512
SCALE = float(1.0 / np.sqrt(DH))
MASK_NEG = -30000.0
EPS = float(np.finfo(np.float32).eps)
ROPE_BASE = 10000.0


def _round_f32r(x: np.ndarray) -> np.ndarray:
    """Round-to-nearest-even fp32 -> fp32r (e8m11, low 12 bits zero)."""
    x = np.ascontiguousarray(x, dtype=np.float32)
    u = x.view(np.uint32).astype(np.uint64)
    lo = u & 0xFFF
    base = u & ~np.uint64(0xFFF)
    lsb = (u >> np.uint64(12)) & 1
    rnd = (lo > 0x800) | ((lo == 0x800) & (lsb == 1))
    out = base + np.where(rnd, np.uint64(0x1000), np.uint64(0))
    return out.astype(np.uint32).view(np.float32).reshape(x.shape)


def _chunk_part_major(a: np.ndarray, chunk: int = P) -> np.ndarray:
    """[C*chunk, N] -> [chunk, C, N] with out[p, c, :] = a[c*chunk + p, :]."""
    c = a.shape[0] // chunk
    return np.ascontiguousarray(a.reshape(c, chunk, a.shape[1]).transpose(1, 0, 2))


def _phase_a(nc, tc, tensors):
    (xt_d, wq_d, wkv_d, wgt_d, v1s_d, cos_d, sin_d,
     qt_t, kt_t, vn_t, gt_t, ident, eps_t) = tensors
    HF = DH // 2
    with (
        tc.tile_pool(name="aw", bufs=1) as aw,
        tc.tile_pool(name="wka", bufs=2) as wka,
        tc.tile_pool(name="scra", bufs=1) as scra,
        tc.tile_pool(name="psa2", bufs=2, space="PSUM") as psa2,
        tc.tile_pool(name="psa1", bufs=1, space="PSUM") as psa1,
    ):
        wq_t = aw.tile([P, NKC, HLOC * DH], F32R, tag="wq")
        wkv_t = aw.tile([P, NKC, 2 * DH], F32R, tag="wkv")
        wgt_t = aw.tile([NH, HLOC], F32, tag="wgt")
        cos_t = aw.tile([P, NT, DH], F32, tag="cos")
        sin_t = aw.tile([P, NT, DH], F32, tag="sin")
        v1_t = aw.tile([P, NT, DH], F32, tag="v1")
        gt4 = aw.tile([HLOC, T], F32, tag="gt4")
        nc.sync.dma_start(wq_t[:], wq_d[:])
        nc.sync.dma_start(wkv_t[:], wkv_d[:])
        nc.sync.dma_start(wgt_t[:], wgt_d[:])
        nc.sync.dma_start(cos_t[:], cos_d[:])
        nc.sync.dma_start(sin_t[:], sin_d[:])
        nc.sync.dma_start(v1_t[:], v1s_d[:])

        for t in range(NT):
            xt_t = wka.tile([P, NKC, P], F32R, tag="xt")
            nc.sync.dma_start(xt_t[:], xt_d[:, :, t * P:(t + 1) * P])

            # gates (one tiny fp32 matmul, k=16, m=4)
            gp = psa1.tile([HLOC, P], F32, tag="gp")
            nc.tensor.matmul(
                gp[:], wgt_t[:], xt_t[0:NH, 0, :].bitcast(F32),
                start=True, stop=True,
            )
            nc.scalar.activation(gt4[:, t * P:(t + 1) * P], gp[:], AF.Sigmoid)

            # Q / KV projections
            q_ps = psa2.tile([P, HLOC * DH], F32, tag="qps")
            for kc in range(NKC):
                nc.tensor.matmul(
                    q_ps[:], xt_t[:, kc, :], wq_t[:, kc, :],
                    start=(kc == 0), stop=(kc == NKC - 1),
                )
            kv_ps = psa2.tile([P, 2 * DH], F32, tag="kvps")
            for kc in range(NKC):
                nc.tensor.matmul(
                    kv_ps[:], xt_t[:, kc, :], wkv_t[:, kc, :],
                    start=(kc == 0), stop=(kc == NKC - 1),
                )

            # V = x@Wv*(1-lamb) + lamb*v1   (Wv prescaled, v1 prescaled)
            nc.vector.tensor_add(vn_t[:, t, :], kv_ps[:, DH:2 * DH], v1_t[:, t, :])

            # rmsnorm + rope on Q (4 heads) and K
            q_sb = wka.tile([P, HLOC * DH], F32, tag="qsb")
            nc.scalar.activation(q_sb[:], q_ps[:], AF.Copy)
            k_sb = wka.tile([P, DH], F32, tag="ksb")
            nc.scalar.activation(k_sb[:], kv_ps[:, 0:DH], AF.Copy)

            ms_t = scra.tile([P, HLOC + 1], F32, tag="ms")
            sq_scr = scra.tile([P, DH], F32, tag="sqscr")
            for h in range(HLOC):
                nc.scalar.activation(
                    sq_scr[:], q_sb[:, h * DH:(h + 1) * DH], AF.Square,
                    accum_out=ms_t[:, h:h + 1],
                )
            nc.scalar.activation(sq_scr[:], k_sb[:], AF.Square,
                                 accum_out=ms_t[:, HLOC:HLOC + 1])
            rt_t = scra.tile([P, HLOC + 1], F32, tag="rt")
            nc.scalar.activation(rt_t[:], ms_t[:], AF.Sqrt,
                                 scale=float(1.0 / DH), bias=eps_t[:, 0:1])
            rs_t = scra.tile([P, HLOC + 1], F32, tag="rs")
            nc.vector.reciprocal(rs_t[:], rt_t[:])

            qc = scra.tile([P, HLOC * DH], F32, tag="qc")
            qs = scra.tile([P, HLOC * DH], F32, tag="qs")
            rot = wka.tile([P, HLOC * DH], F32, tag="rot")
            for h in range(HLOC):
                sl = slice(h * DH, (h + 1) * DH)
                nc.vector.tensor_mul(qc[:, sl], q_sb[:, sl], cos_t[:, t, :])
                nc.vector.tensor_mul(qs[:, sl], q_sb[:, sl], sin_t[:, t, :])
                nc.vector.tensor_sub(
                    rot[:, h * DH:h * DH + HF],
                    qc[:, h * DH:h * DH + HF], qs[:, h * DH + HF:(h + 1) * DH])
                nc.vector.tensor_add(
                    rot[:, h * DH + HF:(h + 1) * DH],
                    qc[:, h * DH + HF:(h + 1) * DH], qs[:, h * DH:h * DH + HF])
                nc.vector.tensor_scalar_mul(rot[:, sl], rot[:, sl], rs_t[:, h:h + 1])
                tp = psa2.tile([P, P], F32, tag="tr")
                nc.tensor.transpose(tp[:], rot[:, sl], ident[:])
                nc.scalar.activation(qt_t[:, h, t * P:(t + 1) * P], tp[:], AF.Copy)

            kc_t = scra.tile([P, DH], F32, tag="kc")
            ks_t = scra.tile([P, DH], F32, tag="ks")
            krot = wka.tile([P, DH], F32, tag="krot")
            nc.vector.tensor_mul(kc_t[:], k_sb[:], cos_t[:, t, :])
            nc.vector.tensor_mul(ks_t[:], k_sb[:], sin_t[:, t, :])
            nc.vector.tensor_sub(krot[:, 0:HF], kc_t[:, 0:HF], ks_t[:, HF:DH])
            nc.vector.tensor_add(krot[:, HF:DH], kc_t[:, HF:DH], ks_t[:, 0:HF])
            nc.vector.tensor_scalar_mul(krot[:], krot[:], rs_t[:, HLOC:HLOC + 1])
            tp = psa2.tile([P, P], F32, tag="tr")
            nc.tensor.transpose(tp[:], krot[:], ident[:])
            nc.scalar.activation(kt_t[:, t * P:(t + 1) * P], tp[:], AF.Copy)

        # extract per-head gate rows to partition-0 tiles (DMA is exempt
        # from the 32-strip partition-base rule)
        for h in range(HLOC):
            nc.sync.dma_start(gt_t[h][:], gt4[h:h + 1, :])


def _phase_b(nc, tc, bw, tensors):
    (msk_d, qt_t, kt_t, vn_t, gt_t, yt_t, onec_r) = tensors
    msk_t = bw.tile([P, NTB, TB], F32, tag="msk")
    nc.sync.dma_start(msk_t[:], msk_d[:])
    with (
        tc.tile_pool(name="wkb", bufs=4) as wkb,
        tc.tile_pool(name="smb", bufs=2) as smb,
        tc.tile_pool(name="psb3", bufs=3, space="PSUM") as psb3,
        tc.tile_pool(name="psb2", bufs=2, space="PSUM") as psb2,
        tc.tile_pool(name="psb1", bufs=1, space="PSUM") as psb1,
    ):
        for h in range(HLOC):
            for tb in range(NTB):
                nj = (tb + 1) * (TB // P)  # s-tiles needed
                y_ps = psb2.tile([P, TB], F32, tag="yps")
                lacc = smb.tile([1, TB], F32, tag="lacc")
                for j in range(nj):
                    s_ps = psb3.tile([P, TB], F32, tag="sps")
                    nc.tensor.matmul(
                        s_ps[:], kt_t[:, j * P:(j + 1) * P],
                        qt_t[:, h, tb * TB:(tb + 1) * TB],
                        start=True, stop=True,
                    )
                    r = j - (tb * (TB // P))
                    if r >= 0:
                        nc.vector.tensor_add(s_ps[:], s_ps[:], msk_t[:, r, :])
                    p_t = wkb.tile([P, TB], F32R, tag="pt")
                    nc.scalar.activation(p_t[:], s_ps[:], AF.Exp, scale=SCALE)
                    nc.tensor.matmul(y_ps[:], vn_t[:, j, :], p_t[:],
                                     start=(j == 0), stop=(j == nj - 1))
                    # softmax denominator partial on GPSIMD (partition sum)
                    lpart = smb.tile([1, TB], F32, tag="lpart")
                    nc.gpsimd.tensor_reduce(
                        lpart[:], p_t[:].bitcast(F32),
                        axis=mybir.AxisListType.C, op=mybir.AluOpType.add)
                    if j == 0:
                        nc.vector.tensor_copy(lacc[:], lpart[:])
                    else:
                        nc.vector.tensor_add(lacc[:], lacc[:], lpart[:])

                linv = smb.tile([1, TB], F32, tag="linv")
                nc.vector.reciprocal(linv[:], lacc[:])
                alpha = smb.tile([1, TB], F32R, tag="alpha")
                nc.vector.tensor_mul(alpha[:], linv[:],
                                     gt_t[h][:, tb * TB:(tb + 1) * TB])
                bc_ps = psb1.tile([P, TB], F32, tag="bcps")
                nc.tensor.matmul(bc_ps[:], onec_r[:], alpha[:],
                                 start=True, stop=True)
                bc_sb = smb.tile([P, TB], F32, tag="bcsb")
                nc.scalar.activation(bc_sb[:], bc_ps[:], AF.Copy)
                nc.vector.tensor_mul(yt_t[:, h, tb * TB:(tb + 1) * TB],
                                     y_ps[:], bc_sb[:])


def _phase_c(nc, tc, out_d, yt_t, wo_t):
    with (
        tc.tile_pool(name="wkc", bufs=3) as wkc,
        tc.tile_pool(name="psc", bufs=2, space="PSUM") as psc,
    ):
        for t in range(NT):
            for dc in range(4):
                o_ps = psc.tile([P, 512], F32, tag="ops")
                for h in range(HLOC):
                    nc.tensor.matmul(
                        o_ps[:], yt_t[:, h, t * P:(t + 1) * P],
                        wo_t[:, h, dc * 512:(dc + 1) * 512],
                        start=(h == 0), stop=(h == HLOC - 1),
                    )
                o_sb = wkc.tile([P, 512], F32, tag="osb")
                nc.scalar.activation(o_sb[:], o_ps[:], AF.Copy)
                nc.sync.dma_start(
                    out_d[t * P:(t + 1) * P, dc * 512:(dc + 1) * 512], o_sb[:])


def _build_nc():
    nc = bacc.Bacc("TRN2", target_bir_lowering=False, debug=False)

    xt_d = nc.dram_tensor("xt", [P, NKC, T], F32R, kind="ExternalInput")
    wq_d = nc.dram_tensor("wq", [P, NKC, HLOC * DH], F32R, kind="ExternalInput")
    wkv_d = nc.dram_tensor("wkv", [P, NKC, 2 * DH], F32R, kind="ExternalInput")
    wo_d = nc.dram_tensor("wo", [P, HLOC, D], F32R, kind="ExternalInput")
    wgt_d = nc.dram_tensor("wgt", [NH, HLOC], F32, kind="ExternalInput")
    v1s_d = nc.dram_tensor("v1s", [P, NT, DH], F32, kind="ExternalInput")
    cos_d = nc.dram_tensor("cos2", [P, NT, DH], F32, kind="ExternalInput")
    sin_d = nc.dram_tensor("sin2", [P, NT, DH], F32, kind="ExternalInput")
    msk_d = nc.dram_tensor("maskt", [P, NTB, TB], F32, kind="ExternalInput")
    out_d = nc.dram_tensor("out", [T, D], F32, kind="ExternalOutput")

    with tile.TileContext(nc) as tc:
        with (
            tc.tile_pool(name="cst", bufs=1) as cst,
            tc.tile_pool(name="res", bufs=1) as res,
        ):
            # ---- constants ----
            ident = cst.tile([P, P], F32, tag="ident")
            make_identity(nc, ident[:])
            onec_f = cst.tile([1, P], F32, tag="onec_f")
            nc.vector.memset(onec_f[:], 1.0)
            onec_r = cst.tile([1, P], F32R, tag="onec_r")
            nc.scalar.activation(onec_r[:], onec_f[:], AF.Copy)
            eps_t = cst.tile([P, 1], F32, tag="eps")
            nc.vector.memset(eps_t[:], EPS)

            # ---- tensors spanning phase A -> B ----
            qt_t = res.tile([P, HLOC, T], F32R, tag="QT")     # Q^T per head
            kt_t = res.tile([P, T], F32R, tag="KT")           # K^T
            vn_t = res.tile([P, NT, DH], F32R, tag="VN")      # V natural (s-tiled)
            gt_t = [res.tile([1, T], F32, tag=f"GT{h}", name=f"GT{h}")
                    for h in range(HLOC)]

            _phase_a(nc, tc, (xt_d, wq_d, wkv_d, wgt_d, v1s_d, cos_d, sin_d,
                              qt_t, kt_t, vn_t, gt_t, ident, eps_t))

            with tc.tile_pool(name="bw", bufs=1) as bw:
                wo_t = bw.tile([P, HLOC, D], F32R, tag="wo")
                nc.sync.dma_start(wo_t[:], wo_d[:])
                yt_t = bw.tile([P, HLOC, T], F32R, tag="YT")  # y^T per head

                _phase_b(nc, tc, bw, (msk_d, qt_t, kt_t, vn_t, gt_t, yt_t,
                                      onec_r))
                _phase_c(nc, tc, out_d, yt_t, wo_t)

    nc.compile()
    return nc


_NC_CACHE = None


def _get_nc():
    global _NC_CACHE
    if _NC_CACHE is None:
        _NC_CACHE = _build_nc()
    return _NC_CACHE


def _make_in_maps(x, pos_ids, v1, Wq, Wk, Wv, Wo, Wg, v_lamb):
    x = np.asarray(x, np.float32)
    pos_ids = np.asarray(pos_ids)
    v1 = np.asarray(v1, np.float32)
    Wq = np.asarray(Wq, np.float32)
    Wk = np.asarray(Wk, np.float32)
    Wv = np.asarray(Wv, np.float32)
    Wo = np.asarray(Wo, np.float32)
    Wg = np.asarray(Wg, np.float32)
    lamb = np.float32(np.asarray(v_lamb))

    # rope tables from pos_ids (fp32 math to match reference)
    half = DH // 2
    inv_freq = (1.0 / (np.float32(ROPE_BASE) **
                       (np.arange(half, dtype=np.float32) / np.float32(half)))
                ).astype(np.float32)
    ang = pos_ids.astype(np.float32)[:, None] * inv_freq[None, :]
    cos = np.cos(ang).astype(np.float32)
    sin = np.sin(ang).astype(np.float32)
    cos2 = _chunk_part_major(np.concatenate([cos, cos], axis=1))
    sin2 = _chunk_part_major(np.concatenate([sin, sin], axis=1))

    # mask variants [P, NTB, TB]: 0 if (c - 128*r) >= i else MASK_NEG
    i_idx = np.arange(P)[:, None, None]
    r_idx = np.arange(NTB)[None, :, None]
    c_idx = np.arange(TB)[None, None, :]
    maskt = np.where((c_idx - P * r_idx) >= i_idx, 0.0, MASK_NEG).astype(np.float32)

    xt_rounded = [
        _chunk_part_major(np.ascontiguousarray(_round_f32r(x[b]).T))
        for b in range(B)
    ]

    in_maps = []
    for c in range(8):
        b, g = divmod(c, 4)
        wq_g = _round_f32r(_chunk_part_major(Wq[:, 4 * g * DH:(4 * g + 4) * DH]))
        wkv = np.concatenate(
            [Wk[:, g * DH:(g + 1) * DH],
             (1.0 - lamb) * Wv[:, g * DH:(g + 1) * DH]], axis=1)
        wkv_g = _round_f32r(_chunk_part_major(wkv))
        wo_g = _round_f32r(_chunk_part_major(Wo[4 * g * DH:(4 * g + 4) * DH, :]))
        wgt_g = np.ascontiguousarray(Wg[4 * g:4 * g + 4, :].T)
        v1s_g = _chunk_part_major(lamb * v1[b, g])
        in_maps.append({
            "xt": xt_rounded[b],
            "wq": wq_g, "wkv": wkv_g, "wo": wo_g, "wgt": wgt_g,
            "v1s": v1s_g, "cos2": cos2, "sin2": sin2, "maskt": maskt,
        })
    return in_maps


def kernel(x, pos_ids, v1, Wq, Wk, Wv, Wo, Wg, v_lamb,
           _trace=False, _res_out=None, _tmpdir=None):
    nc = _get_nc()
    in_maps = _make_in_maps(x, pos_ids, v1, Wq, Wk, Wv, Wo, Wg, v_lamb)
    res = run_bass_kernel_spmd(nc, in_maps, list(range(8)), trace=_trace,
                               tmpdir=_tmpdir)
    if _res_out is not None:
        _res_out.append(res)
    out = np.zeros((B, T, D), np.float32)
    for c in range(8):
        b = c // 4
        out[b] += res.results[c]["out"]
    return out, np.asarray(v1, np.float32)


# revision 12
# speedup vs baseline: 18.3353x; 18.3353x over previous
"""Fused causal GQA attention block (RMSNorm+RoPE+value-residual+gated attn)
for Trainium2, SPMD over 8 NeuronCores.

Sharding: core c = b*4 + g  (b in {0,1} batch, g in {0..3} kv-head group).
Each core computes its batch's 4 q-heads (one kv head) end-to-end:
  Q/K/V projections, RMSNorm+RoPE, causal softmax (no max-sub needed:
  |scores| <= sqrt(128)), value-residual lerp (folded into Wv/v1 on host),
  sigmoid gating, and a partial output projection (its 512 rows of Wo).
Host sums the 4 partial out-projections per batch.

x is shipped pre-transposed (X^T) so no on-device transposition of the
activations is needed; all matmuls run as float32r (e8m11) on the PE at
bf16 speed. Softmax denominators are partition-reduced on the (otherwise
idle) GPSIMD engine instead of PE ones-matmuls.
"""

import os
import sys
import numpy as np

for _p in ("/opt/trn_rl_repo", "/root/.axon_site/_ro/trn_rl_repo"):
    if os.path.isdir(_p) and _p not in sys.path:
        sys.path.insert(0, _p)

import concourse.bass as bass  # noqa: E402
import concourse.mybir as mybir  # noqa: E402
import concourse.tile as tile  # noqa: E402
from concourse import bacc  # noqa: E402
from concourse.bass_utils import run_bass_kernel_spmd  # noqa: E402
from concourse.masks import make_identity  # noqa: E402

AF = mybir.ActivationFunctionType
F32 = mybir.dt.float32
F32R = mybir.dt.float32r

B, T, D = 2, 2048, 2048
NH, NKV, DH = 16, 4, 128
HLOC = NH // NKV          # 4 q heads per core
P = 128                   # partition tile
NT = T // P               # 16 token tiles
NKC = D // P              # 16 contraction chunks
NTB = 4                   # tq blocks of 512
TB = T // NTB             # 512
SCALE = float(1.0 / np.sqrt(DH))
MASK_NEG = -30000.0
EPS = float(np.finfo(np.float32).eps)
ROPE_BASE = 10000.0


def _round_f32r(x: np.ndarray) -> np.ndarray:
    """Round-to-nearest-even fp32 -> fp32r (e8m11, low 12 bits zero)."""
    x = np.ascontiguousarray(x, dtype=np.float32)
    u = x.view(np.uint32).astype(np.uint64)
    lo = u & 0xFFF
    base = u & ~np.uint64(0xFFF)
    lsb = (u >> np.uint64(12)) & 1
    rnd = (lo > 0x800) | ((lo == 0x800) & (lsb == 1))
    out = base + np.where(rnd, np.uint64(0x1000), np.uint64(0))
    return out.astype(np.uint32).view(np.float32).reshape(x.shape)


def _chunk_part_major(a: np.ndarray, chunk: int = P) -> np.ndarray:
    """[C*chunk, N] -> [chunk, C, N] with out[p, c, :] = a[c*chunk + p, :]."""
    c = a.shape[0] // chunk
    return np.ascontiguousarray(a.reshape(c, chunk, a.shape[1]).transpose(1, 0, 2))


def _phase_a(nc, tc, tensors):
    (xt_d, wq_d, wkv_d, wgt_d, v1s_d, cos_d, sin_d,
     qt_t, kt_t, vn_t, gt_t, ident, eps_t) = tensors
    HF = DH // 2
    with (
        tc.tile_pool(name="aw", bufs=1) as aw,
        tc.tile_pool(name="wka", bufs=2) as wka,
        tc.tile_pool(name="scra", bufs=1) as scra,
        tc.tile_pool(name="psa2", bufs=2, space="PSUM") as psa2,
        tc.tile_pool(name="psa1", bufs=1, space="PSUM") as psa1,
    ):
        wq_t = aw.tile([P, NKC, HLOC * DH], F32R, tag="wq")
        wkv_t = aw.tile([P, NKC, 2 * DH], F32R, tag="wkv")
        wgt_t = aw.tile([NH, HLOC], F32, tag="wgt")
        cos_t = aw.tile([P, NT, DH], F32, tag="cos")
        sin_t = aw.tile([P, NT, DH], F32, tag="sin")
        v1_t = aw.tile([P, NT, DH], F32, tag="v1")
        gt4 = aw.tile([HLOC, T], F32, tag="gt4")
        nc.sync.dma_start(wq_t[:], wq_d[:])
        nc.sync.dma_start(wkv_t[:], wkv_d[:])
        nc.sync.dma_start(wgt_t[:], wgt_d[:])
        nc.sync.dma_start(cos_t[:], cos_d[:])
        nc.sync.dma_start(sin_t[:], sin_d[:])
        nc.sync.dma_start(v1_t[:], v1s_d[:])

        for t in range(NT):
            xt_t = wka.tile([P, NKC, P], F32R, tag="xt")
            nc.sync.dma_start(xt_t[:], xt_d[:, :, t * P:(t + 1) * P])

            # gates (one tiny fp32 matmul, k=16, m=4)
            gp = psa1.tile([HLOC, P], F32, tag="gp")
            nc.tensor.matmul(
                gp[:], wgt_t[:], xt_t[0:NH, 0, :].bitcast(F32),
                start=True, stop=True,
            )
            nc.scalar.activation(gt4[:, t * P:(t + 1) * P], gp[:], AF.Sigmoid)

            # Q / KV projections
            q_ps = psa2.tile([P, HLOC * DH], F32, tag="qps")
            for kc in range(NKC):
                nc.tensor.matmul(
                    q_ps[:], xt_t[:, kc, :], wq_t[:, kc, :],
                    start=(kc == 0), stop=(kc == NKC - 1),
                )
            kv_ps = psa2.tile([P, 2 * DH], F32, tag="kvps")
            for kc in range(NKC):
                nc.tensor.matmul(
                    kv_ps[:], xt_t[:, kc, :], wkv_t[:, kc, :],
                    start=(kc == 0), stop=(kc == NKC - 1),
                )

            # V = x@Wv*(1-lamb) + lamb*v1   (Wv prescaled, v1 prescaled)
            nc.vector.tensor_add(vn_t[:, t, :], kv_ps[:, DH:2 * DH], v1_t[:, t, :])

            # rmsnorm + rope on Q (4 heads) and K
            q_sb = wka.tile([P, HLOC * DH], F32, tag="qsb")
            nc.scalar.activation(q_sb[:], q_ps[:], AF.Copy)
            k_sb = wka.tile([P, DH], F32, tag="ksb")
            nc.scalar.activation(k_sb[:], kv_ps[:, 0:DH], AF.Copy)

            ms_t = scra.tile([P, HLOC + 1], F32, tag="ms")
            sq_scr = scra.tile([P, DH], F32, tag="sqscr")
            for h in range(HLOC):
                nc.scalar.activation(
                    sq_scr[:], q_sb[:, h * DH:(h + 1) * DH], AF.Square,
                    accum_out=ms_t[:, h:h + 1],
                )
            nc.scalar.activation(sq_scr[:], k_sb[:], AF.Square,
                                 accum_out=ms_t[:, HLOC:HLOC + 1])
            rt_t = scra.tile([P, HLOC + 1], F32, tag="rt")
            nc.scalar.activation(rt_t[:], ms_t[:], AF.Sqrt,
                                 scale=float(1.0 / DH), bias=eps_t[:, 0:1])
            rs_t = scra.tile([P, HLOC + 1], F32, tag="rs")
            nc.vector.reciprocal(rs_t[:], rt_t[:])

            qc = scra.tile([P, HLOC * DH], F32, tag="qc")
            qs = scra.tile([P, HLOC * DH], F32, tag="qs")
            rot = wka.tile([P, HLOC * DH], F32, tag="rot")
            for h in range(HLOC):
                sl = slice(h * DH, (h + 1) * DH)
                nc.vector.tensor_mul(qc[:, sl], q_sb[:, sl], cos_t[:, t, :])
                nc.vector.tensor_mul(qs[:, sl], q_sb[:, sl], sin_t[:, t, :])
                nc.vector.tensor_sub(
                    rot[:, h * DH:h * DH + HF],
                    qc[:, h * DH:h * DH + HF], qs[:, h * DH + HF:(h + 1) * DH])
                nc.vector.tensor_add(
                    rot[:, h * DH + HF:(h + 1) * DH],
                    qc[:, h * DH + HF:(h + 1) * DH], qs[:, h * DH:h * DH + HF])
                nc.vector.tensor_scalar_mul(rot[:, sl], rot[:, sl], rs_t[:, h:h + 1])
                tp = psa2.tile([P, P], F32, tag="tr")
                nc.tensor.transpose(tp[:], rot[:, sl], ident[:])
                nc.scalar.activation(qt_t[:, h, t * P:(t + 1) * P], tp[:], AF.Copy)

            kc_t = scra.tile([P, DH], F32, tag="kc")
            ks_t = scra.tile([P, DH], F32, tag="ks")
            krot = wka.tile([P, DH], F32, tag="krot")
            nc.vector.tensor_mul(kc_t[:], k_sb[:], cos_t[:, t, :])
            nc.vector.tensor_mul(ks_t[:], k_sb[:], sin_t[:, t, :])
            nc.vector.tensor_sub(krot[:, 0:HF], kc_t[:, 0:HF], ks_t[:, HF:DH])
            nc.vector.tensor_add(krot[:, HF:DH], kc_t[:, HF:DH], ks_t[:, 0:HF])
            nc.vector.tensor_scalar_mul(krot[:], krot[:], rs_t[:, HLOC:HLOC + 1])
            tp = psa2.tile([P, P], F32, tag="tr")
            nc.tensor.transpose(tp[:], krot[:], ident[:])
            nc.scalar.activation(kt_t[:, t * P:(t + 1) * P], tp[:], AF.Copy)

        # extract per-head gate rows to partition-0 tiles (DMA is exempt
        # from the 32-strip partition-base rule)
        for h in range(HLOC):
            nc.sync.dma_start(gt_t[h][:], gt4[h:h + 1, :])


def _phase_b(nc, tc, bw, tensors):
    (msk_d, qt_t, kt_t, vn_t, gt_t, yt_t, onec_r, ones_r) = tensors
    msk_t = bw.tile([P, NTB, TB], F32, tag="msk")
    nc.sync.dma_start(msk_t[:], msk_d[:])
    with (
        tc.tile_pool(name="wkb", bufs=4) as wkb,
        tc.tile_pool(name="smb", bufs=2) as smb,
        tc.tile_pool(name="psb3", bufs=3, space="PSUM") as psb3,
        tc.tile_pool(name="psb2", bufs=2, space="PSUM") as psb2,
        tc.tile_pool(name="psb1", bufs=1, space="PSUM") as psb1,
    ):
        for h in range(HLOC):
            for tb in range(NTB):
                nj = (tb + 1) * (TB // P)  # s-tiles needed
                y_ps = psb2.tile([P, TB], F32, tag="yps")
                l_ps = psb1.tile([1, TB], F32, tag="lps")
                for j in range(nj):
                    s_ps = psb3.tile([P, TB], F32, tag="sps")
                    nc.tensor.matmul(
                        s_ps[:], kt_t[:, j * P:(j + 1) * P],
                        qt_t[:, h, tb * TB:(tb + 1) * TB],
                        start=True, stop=True,
                    )
                    r = j - (tb * (TB // P))
                    if r >= 0:
                        nc.vector.tensor_add(s_ps[:], s_ps[:], msk_t[:, r, :])
                    p_t = wkb.tile([P, TB], F32R, tag="pt")
                    nc.scalar.activation(p_t[:], s_ps[:], AF.Exp, scale=SCALE)
                    nc.tensor.matmul(y_ps[:], vn_t[:, j, :], p_t[:],
                                     start=(j == 0), stop=(j == nj - 1))
                    nc.tensor.matmul(l_ps[:], ones_r[:], p_t[:],
                                     start=(j == 0), stop=(j == nj - 1))

                linv = smb.tile([1, TB], F32, tag="linv")
                nc.vector.reciprocal(linv[:], l_ps[:])
                alpha = smb.tile([1, TB], F32R, tag="alpha")
                nc.vector.tensor_mul(alpha[:], linv[:],
                                     gt_t[h][:, tb * TB:(tb + 1) * TB])
                bc_ps = psb1.tile([P, TB], F32, tag="bcps")
                nc.tensor.matmul(bc_ps[:], onec_r[:], alpha[:],
                                 start=True, stop=True)
                bc_sb = smb.tile([P, TB], F32, tag="bcsb")
                nc.scalar.activation(bc_sb[:], bc_ps[:], AF.Copy)
                nc.vector.tensor_mul(yt_t[:, h, tb * TB:(tb + 1) * TB],
                                     y_ps[:], bc_sb[:])


def _phase_c(nc, tc, out_d, yt_t, wo_t):
    with (
        tc.tile_pool(name="wkc", bufs=3) as wkc,
        tc.tile_pool(name="psc", bufs=2, space="PSUM") as psc,
    ):
        for t in range(NT):
            for dc in range(4):
                o_ps = psc.tile([P, 512], F32, tag="ops")
                for h in range(HLOC):
                    nc.tensor.matmul(
                        o_ps[:], yt_t[:, h, t * P:(t + 1) * P],
                        wo_t[:, h, dc * 512:(dc + 1) * 512],
                        start=(h == 0), stop=(h == HLOC - 1),
                    )
                o_sb = wkc.tile([P, 512], F32, tag="osb")
                nc.scalar.activation(o_sb[:], o_ps[:], AF.Copy)
                nc.sync.dma_start(
                    out_d[t * P:(t + 1) * P, dc * 512:(dc + 1) * 512], o_sb[:])


def _build_nc():
    nc = bacc.Bacc("TRN2", target_bir_lowering=False, debug=False)

    xt_d = nc.dram_tensor("xt", [P, NKC, T], F32R, kind="ExternalInput")
    wq_d = nc.dram_tensor("wq", [P, NKC, HLOC * DH], F32R, kind="ExternalInput")
    wkv_d = nc.dram_tensor("wkv", [P, NKC, 2 * DH], F32R, kind="ExternalInput")
    wo_d = nc.dram_tensor("wo", [P, HLOC, D], F32R, kind="ExternalInput")
    wgt_d = nc.dram_tensor("wgt", [NH, HLOC], F32, kind="ExternalInput")
    v1s_d = nc.dram_tensor("v1s", [P, NT, DH], F32, kind="ExternalInput")
    cos_d = nc.dram_tensor("cos2", [P, NT, DH], F32, kind="ExternalInput")
    sin_d = nc.dram_tensor("sin2", [P, NT, DH], F32, kind="ExternalInput")
    msk_d = nc.dram_tensor("maskt", [P, NTB, TB], F32, kind="ExternalInput")
    out_d = nc.dram_tensor("out", [T, D], F32, kind="ExternalOutput")

    with tile.TileContext(nc) as tc:
        with (
            tc.tile_pool(name="cst", bufs=1) as cst,
            tc.tile_pool(name="res", bufs=1) as res,
        ):
            # ---- constants ----
            ident = cst.tile([P, P], F32, tag="ident")
            make_identity(nc, ident[:])
            ones_f = cst.tile([P, 1], F32, tag="ones_f")
            nc.vector.memset(ones_f[:], 1.0)
            ones_r = cst.tile([P, 1], F32R, tag="ones_r")
            nc.scalar.activation(ones_r[:], ones_f[:], AF.Copy)
            onec_f = cst.tile([1, P], F32, tag="onec_f")
            nc.vector.memset(onec_f[:], 1.0)
            onec_r = cst.tile([1, P], F32R, tag="onec_r")
            nc.scalar.activation(onec_r[:], onec_f[:], AF.Copy)
            eps_t = cst.tile([P, 1], F32, tag="eps")
            nc.vector.memset(eps_t[:], EPS)

            # ---- tensors spanning phase A -> B ----
            qt_t = res.tile([P, HLOC, T], F32R, tag="QT")     # Q^T per head
            kt_t = res.tile([P, T], F32R, tag="KT")           # K^T
            vn_t = res.tile([P, NT, DH], F32R, tag="VN")      # V natural (s-tiled)
            gt_t = [res.tile([1, T], F32, tag=f"GT{h}", name=f"GT{h}")
                    for h in range(HLOC)]

            _phase_a(nc, tc, (xt_d, wq_d, wkv_d, wgt_d, v1s_d, cos_d, sin_d,
                              qt_t, kt_t, vn_t, gt_t, ident, eps_t))

            with tc.tile_pool(name="bw", bufs=1) as bw:
                wo_t = bw.tile([P, HLOC, D], F32R, tag="wo")
                nc.sync.dma_start(wo_t[:], wo_d[:])
                yt_t = bw.tile([P, HLOC, T], F32R, tag="YT")  # y^T per head

                _phase_b(nc, tc, bw, (msk_d, qt_t, kt_t, vn_t, gt_t, yt_t,
                                      onec_r, ones_r))
                _phase_c(nc, tc, out_d, yt_t, wo_t)

    nc.compile()
    return nc


_NC_CACHE = None


def _get_nc():
    global _NC_CACHE
    if _NC_CACHE is None:
        _NC_CACHE = _build_nc()
    return _NC_CACHE


def _make_in_maps(x, pos_ids, v1, Wq, Wk, Wv, Wo, Wg, v_lamb):
    x = np.asarray(x, np.float32)
    pos_ids = np.asarray(pos_ids)
    v1 = np.asarray(v1, np.float32)
    Wq = np.asarray(Wq, np.float32)
    Wk = np.asarray(Wk, np.float32)
    Wv = np.asarray(Wv, np.float32)
    Wo = np.asarray(Wo, np.float32)
    Wg = np.asarray(Wg, np.float32)
    lamb = np.float32(np.asarray(v_lamb))

    # rope tables from pos_ids (fp32 math to match reference)
    half = DH // 2
    inv_freq = (1.0 / (np.float32(ROPE_BASE) **
                       (np.arange(half, dtype=np.float32) / np.float32(half)))
                ).astype(np.float32)
    ang = pos_ids.astype(np.float32)[:, None] * inv_freq[None, :]
    cos = np.cos(ang).astype(np.float32)
    sin = np.sin(ang).astype(np.float32)
    cos2 = _chunk_part_major(np.concatenate([cos, cos], axis=1))
    sin2 = _chunk_part_major(np.concatenate([sin, sin], axis=1))

    # mask variants [P, NTB, TB]: 0 if (c - 128*r) >= i else MASK_NEG
    i_idx = np.arange(P)[:, None, None]
    r_idx = np.arange(NTB)[None, :, None]
    c_idx = np.arange(TB)[None, None, :]
    maskt = np.where((c_idx - P * r_idx) >= i_idx, 0.0, MASK_NEG).astype(np.float32)

    xt_rounded = [
        _chunk_part_major(np.ascontiguousarray(_round_f32r(x[b]).T))
        for b in range(B)
    ]

    in_maps = []
    for c in range(8):
        b, g = divmod(c, 4)
        wq_g = _round_f32r(_chunk_part_major(Wq[:, 4 * g * DH:(4 * g + 4) * DH]))
        wkv = np.concatenate(
            [Wk[:, g * DH:(g + 1) * DH],
             (1.0 - lamb) * Wv[:, g * DH:(g + 1) * DH]], axis=1)
        wkv_g = _round_f32r(_chunk_part_major(wkv))
        wo_g = _round_f32r(_chunk_part_major(Wo[4 * g * DH:(4 * g + 4) * DH, :]))
        wgt_g = np.ascontiguousarray(Wg[4 * g:4 * g + 4, :].T)
        v1s_g = _chunk_part_major(lamb * v1[b, g])
        in_maps.append({
            "xt": xt_rounded[b],
            "wq": wq_g, "wkv": wkv_g, "wo": wo_g, "wgt": wgt_g,
            "v1s": v1s_g, "cos2": cos2, "sin2": sin2, "maskt": maskt,
        })
    return in_maps


def kernel(x, pos_ids, v1, Wq, Wk, Wv, Wo, Wg, v_lamb,
           _trace=False, _res_out=None, _tmpdir=None):
    nc = _get_nc()
    in_maps = _make_in_maps(x, pos_ids, v1, Wq, Wk, Wv, Wo, Wg, v_lamb)
    res = run_bass_kernel_spmd(nc, in_maps, list(range(8)), trace=_trace,
                               tmpdir=_tmpdir)
    if _res_out is not None:
        _res_out.append(res)
    out = np.zeros((B, T, D), np.float32)
    for c in range(8):
        b = c // 4
        out[b] += res.results[c]["out"]
    return out, np.asarray(v1, np.float32)


# revision 13
# speedup vs baseline: 21.7821x; 1.1880x over previous
"""Fused causal GQA attention block (RMSNorm+RoPE+value-residual+gated attn)
for Trainium2, SPMD over 8 NeuronCores.

Sharding: core c = b*4 + g  (b in {0,1} batch, g in {0..3} kv-head group).
Each core computes its batch's 4 q-heads (one kv head) end-to-end:
  Q/K/V projections, RMSNorm+RoPE, causal softmax (no max-sub needed:
  |scores| <= sqrt(128)), value-residual lerp (folded into Wv/v1 on host),
  sigmoid gating, and a partial output projection (its 512 rows of Wo).
Host sums the 4 partial out-projections per batch.

x is shipped pre-transposed (X^T) so no on-device transposition of the
activations is needed; all matmuls run as float32r (e8m11) on the PE at
bf16 speed. Softmax denominators are partition-reduced on the (otherwise
idle) GPSIMD engine instead of PE ones-matmuls.
"""

import os
import sys
import numpy as np

for _p in ("/opt/trn_rl_repo", "/root/.axon_site/_ro/trn_rl_repo"):
    if os.path.isdir(_p) and _p not in sys.path:
        sys.path.insert(0, _p)

import concourse.bass as bass  # noqa: E402
import concourse.mybir as mybir  # noqa: E402
import concourse.tile as tile  # noqa: E402
from concourse import bacc  # noqa: E402
from concourse.bass_utils import run_bass_kernel_spmd  # noqa: E402
from concourse.masks import make_identity  # noqa: E402

AF = mybir.ActivationFunctionType
F32 = mybir.dt.float32
F32R = mybir.dt.float32r

B, T, D = 2, 2048, 2048
NH, NKV, DH = 16, 4, 128
HLOC = NH // NKV          # 4 q heads per core
P = 128                   # partition tile
NT = T // P               # 16 token tiles
NKC = D // P              # 16 contraction chunks
NTB = 4                   # tq blocks of 512
TB = T // NTB             # 512
SCALE = float(1.0 / np.sqrt(DH))
MASK_NEG = -30000.0
EPS = float(np.finfo(np.float32).eps)
ROPE_BASE = 10000.0


def _round_f32r(x: np.ndarray) -> np.ndarray:
    """Round-to-nearest-even fp32 -> fp32r (e8m11, low 12 bits zero)."""
    x = np.ascontiguousarray(x, dtype=np.float32)
    u = x.view(np.uint32).astype(np.uint64)
    lo = u & 0xFFF
    base = u & ~np.uint64(0xFFF)
    lsb = (u >> np.uint64(12)) & 1
    rnd = (lo > 0x800) | ((lo == 0x800) & (lsb == 1))
    out = base + np.where(rnd, np.uint64(0x1000), np.uint64(0))
    return out.astype(np.uint32).view(np.float32).reshape(x.shape)


def _chunk_part_major(a: np.ndarray, chunk: int = P) -> np.ndarray:
    """[C*chunk, N] -> [chunk, C, N] with out[p, c, :] = a[c*chunk + p, :]."""
    c = a.shape[0] // chunk
    return np.ascontiguousarray(a.reshape(c, chunk, a.shape[1]).transpose(1, 0, 2))


def _phase_a(nc, tc, tensors):
    (xt_d, wq_d, wkv_d, wgt_d, v1s_d, cos_d, sin_d,
     qt_t, kt_t, vn_t, gt_t, ident, eps_t) = tensors
    HF = DH // 2
    with (
        tc.tile_pool(name="aw", bufs=1) as aw,
        tc.tile_pool(name="wka", bufs=2) as wka,
        tc.tile_pool(name="scra", bufs=1) as scra,
        tc.tile_pool(name="psa2", bufs=2, space="PSUM") as psa2,
        tc.tile_pool(name="psa1", bufs=1, space="PSUM") as psa1,
    ):
        wq_t = aw.tile([P, NKC, HLOC * DH], F32R, tag="wq")
        wkv_t = aw.tile([P, NKC, 2 * DH], F32R, tag="wkv")
        wgt_t = aw.tile([NH, HLOC], F32, tag="wgt")
        cos_t = aw.tile([P, NT, DH], F32, tag="cos")
        sin_t = aw.tile([P, NT, DH], F32, tag="sin")
        v1_t = aw.tile([P, NT, DH], F32, tag="v1")
        gt4 = aw.tile([HLOC, T], F32, tag="gt4")
        nc.sync.dma_start(wq_t[:], wq_d[:])
        nc.sync.dma_start(wkv_t[:], wkv_d[:])
        nc.sync.dma_start(wgt_t[:], wgt_d[:])
        nc.sync.dma_start(cos_t[:], cos_d[:])
        nc.sync.dma_start(sin_t[:], sin_d[:])
        nc.sync.dma_start(v1_t[:], v1s_d[:])

        for t in range(NT):
            xt_t = wka.tile([P, NKC, P], F32R, tag="xt")
            nc.sync.dma_start(xt_t[:], xt_d[:, :, t * P:(t + 1) * P])

            # gates (one tiny fp32 matmul, k=16, m=4)
            gp = psa1.tile([HLOC, P], F32, tag="gp")
            nc.tensor.matmul(
                gp[:], wgt_t[:], xt_t[0:NH, 0, :].bitcast(F32),
                start=True, stop=True,
            )
            nc.scalar.activation(gt4[:, t * P:(t + 1) * P], gp[:], AF.Sigmoid)

            # Q / KV projections
            q_ps = psa2.tile([P, HLOC * DH], F32, tag="qps")
            for kc in range(NKC):
                nc.tensor.matmul(
                    q_ps[:], xt_t[:, kc, :], wq_t[:, kc, :],
                    start=(kc == 0), stop=(kc == NKC - 1),
                )
            kv_ps = psa2.tile([P, 2 * DH], F32, tag="kvps")
            for kc in range(NKC):
                nc.tensor.matmul(
                    kv_ps[:], xt_t[:, kc, :], wkv_t[:, kc, :],
                    start=(kc == 0), stop=(kc == NKC - 1),
                )

            # V = x@Wv*(1-lamb) + lamb*v1   (Wv prescaled, v1 prescaled)
            nc.vector.tensor_add(vn_t[:, t, :], kv_ps[:, DH:2 * DH], v1_t[:, t, :])

            # rmsnorm + rope on Q (4 heads) and K
            q_sb = wka.tile([P, HLOC * DH], F32, tag="qsb")
            nc.scalar.activation(q_sb[:], q_ps[:], AF.Copy)
            k_sb = wka.tile([P, DH], F32, tag="ksb")
            nc.scalar.activation(k_sb[:], kv_ps[:, 0:DH], AF.Copy)

            ms_t = scra.tile([P, HLOC + 1], F32, tag="ms")
            sq_scr = scra.tile([P, DH], F32, tag="sqscr")
            for h in range(HLOC):
                nc.scalar.activation(
                    sq_scr[:], q_sb[:, h * DH:(h + 1) * DH], AF.Square,
                    accum_out=ms_t[:, h:h + 1],
                )
            nc.scalar.activation(sq_scr[:], k_sb[:], AF.Square,
                                 accum_out=ms_t[:, HLOC:HLOC + 1])
            rt_t = scra.tile([P, HLOC + 1], F32, tag="rt")
            nc.scalar.activation(rt_t[:], ms_t[:], AF.Sqrt,
                                 scale=float(1.0 / DH), bias=eps_t[:, 0:1])
            rs_t = scra.tile([P, HLOC + 1], F32, tag="rs")
            nc.vector.reciprocal(rs_t[:], rt_t[:])

            qc = scra.tile([P, HLOC * DH], F32, tag="qc")
            qs = scra.tile([P, HLOC * DH], F32, tag="qs")
            rot = wka.tile([P, HLOC * DH], F32, tag="rot")
            for h in range(HLOC):
                sl = slice(h * DH, (h + 1) * DH)
                nc.vector.tensor_mul(qc[:, sl], q_sb[:, sl], cos_t[:, t, :])
                nc.vector.tensor_mul(qs[:, sl], q_sb[:, sl], sin_t[:, t, :])
                nc.vector.tensor_sub(
                    rot[:, h * DH:h * DH + HF],
                    qc[:, h * DH:h * DH + HF], qs[:, h * DH + HF:(h + 1) * DH])
                nc.vector.tensor_add(
                    rot[:, h * DH + HF:(h + 1) * DH],
                    qc[:, h * DH + HF:(h + 1) * DH], qs[:, h * DH:h * DH + HF])
                nc.vector.tensor_scalar_mul(rot[:, sl], rot[:, sl], rs_t[:, h:h + 1])
                tp = psa2.tile([P, P], F32, tag="tr")
                nc.tensor.transpose(tp[:], rot[:, sl], ident[:])
                nc.scalar.activation(qt_t[:, h, t * P:(t + 1) * P], tp[:], AF.Copy)

            kc_t = scra.tile([P, DH], F32, tag="kc")
            ks_t = scra.tile([P, DH], F32, tag="ks")
            krot = wka.tile([P, DH], F32, tag="krot")
            nc.vector.tensor_mul(kc_t[:], k_sb[:], cos_t[:, t, :])
            nc.vector.tensor_mul(ks_t[:], k_sb[:], sin_t[:, t, :])
            nc.vector.tensor_sub(krot[:, 0:HF], kc_t[:, 0:HF], ks_t[:, HF:DH])
            nc.vector.tensor_add(krot[:, HF:DH], kc_t[:, HF:DH], ks_t[:, 0:HF])
            nc.vector.tensor_scalar_mul(krot[:], krot[:], rs_t[:, HLOC:HLOC + 1])
            tp = psa2.tile([P, P], F32, tag="tr")
            nc.tensor.transpose(tp[:], krot[:], ident[:])
            nc.scalar.activation(kt_t[:, t * P:(t + 1) * P], tp[:], AF.Copy)

        # extract per-head gate rows to partition-0 tiles (DMA is exempt
        # from the 32-strip partition-base rule)
        for h in range(HLOC):
            nc.sync.dma_start(gt_t[h][:], gt4[h:h + 1, :])


def _phase_b(nc, tc, bw, tensors):
    (msk_d, qt_t, kt_t, vn_t, gt_t, yt_t, onec_r, ones_r) = tensors
    msk_t = bw.tile([P, NTB, TB], F32, tag="msk")
    nc.sync.dma_start(msk_t[:], msk_d[:])
    LOOKAHEAD = 2      # S/exp emitted this many steps ahead of y/l
    EPI_DELAY = 3      # epilogue emitted this many steps after block end
    with (
        tc.tile_pool(name="wkb", bufs=4) as wkb,
        tc.tile_pool(name="smb", bufs=2) as smb,
        tc.tile_pool(name="psb3", bufs=3, space="PSUM") as psb3,
        tc.tile_pool(name="psb2", bufs=2, space="PSUM") as psb2,
        tc.tile_pool(name="psb1", bufs=1, space="PSUM") as psb1,
    ):
        triples = []
        for h in range(HLOC):
            for tb in range(NTB):
                nj = (tb + 1) * (TB // P)
                for j in range(nj):
                    triples.append((h, tb, j, nj))
        N = len(triples)
        pt_live = {}
        blk = {}           # (h, tb) -> dict(y_ps=..., l_ps=...)
        epi_queue = []     # (due_step, h, tb)

        def emit_s(i):
            h, tb, j, nj = triples[i]
            s_ps = psb3.tile([P, TB], F32, tag="sps")
            nc.tensor.matmul(
                s_ps[:], kt_t[:, j * P:(j + 1) * P],
                qt_t[:, h, tb * TB:(tb + 1) * TB],
                start=True, stop=True,
            )
            r = j - (tb * (TB // P))
            if r >= 0:
                nc.vector.tensor_add(s_ps[:], s_ps[:], msk_t[:, r, :])
            p_t = wkb.tile([P, TB], F32R, tag="pt")
            nc.scalar.activation(p_t[:], s_ps[:], AF.Exp, scale=SCALE)
            pt_live[i] = p_t

        def emit_yl(i):
            h, tb, j, nj = triples[i]
            if j == 0:
                y_ps = psb2.tile([P, TB], F32, tag="yps")
                l_ps = psb1.tile([1, TB], F32, tag="lps", bufs=2)
                blk[(h, tb)] = (y_ps, l_ps)
            y_ps, l_ps = blk[(h, tb)]
            p_t = pt_live.pop(i)
            nc.tensor.matmul(y_ps[:], vn_t[:, j, :], p_t[:],
                             start=(j == 0), stop=(j == nj - 1))
            nc.tensor.matmul(l_ps[:], ones_r[:], p_t[:],
                             start=(j == 0), stop=(j == nj - 1))

        def emit_epilogue(h, tb):
            y_ps, l_ps = blk.pop((h, tb))
            linv = smb.tile([1, TB], F32, tag="linv")
            nc.vector.reciprocal(linv[:], l_ps[:])
            alpha = smb.tile([1, TB], F32R, tag="alpha")
            nc.vector.tensor_mul(alpha[:], linv[:],
                                 gt_t[h][:, tb * TB:(tb + 1) * TB])
            bc_ps = psb1.tile([P, TB], F32, tag="bcps")
            nc.tensor.matmul(bc_ps[:], onec_r[:], alpha[:],
                             start=True, stop=True)
            bc_sb = smb.tile([P, TB], F32, tag="bcsb")
            nc.scalar.activation(bc_sb[:], bc_ps[:], AF.Copy)
            nc.vector.tensor_mul(yt_t[:, h, tb * TB:(tb + 1) * TB],
                                 y_ps[:], bc_sb[:])

        for step in range(N + LOOKAHEAD):
            while epi_queue and epi_queue[0][0] <= step:
                _, eh, etb = epi_queue.pop(0)
                emit_epilogue(eh, etb)
            if step < N:
                emit_s(step)
            if step >= LOOKAHEAD:
                i = step - LOOKAHEAD
                emit_yl(i)
                h, tb, j, nj = triples[i]
                if j == nj - 1:
                    epi_queue.append((step + EPI_DELAY, h, tb))
        while epi_queue:
            _, eh, etb = epi_queue.pop(0)
            emit_epilogue(eh, etb)


def _phase_c(nc, tc, out_d, yt_t, wo_t):
    with (
        tc.tile_pool(name="wkc", bufs=3) as wkc,
        tc.tile_pool(name="psc", bufs=2, space="PSUM") as psc,
    ):
        for t in range(NT):
            for dc in range(4):
                o_ps = psc.tile([P, 512], F32, tag="ops")
                for h in range(HLOC):
                    nc.tensor.matmul(
                        o_ps[:], yt_t[:, h, t * P:(t + 1) * P],
                        wo_t[:, h, dc * 512:(dc + 1) * 512],
                        start=(h == 0), stop=(h == HLOC - 1),
                    )
                o_sb = wkc.tile([P, 512], F32, tag="osb")
                nc.scalar.activation(o_sb[:], o_ps[:], AF.Copy)
                nc.sync.dma_start(
                    out_d[t * P:(t + 1) * P, dc * 512:(dc + 1) * 512], o_sb[:])


def _build_nc():
    nc = bacc.Bacc("TRN2", target_bir_lowering=False, debug=False)

    xt_d = nc.dram_tensor("xt", [P, NKC, T], F32R, kind="ExternalInput")
    wq_d = nc.dram_tensor("wq", [P, NKC, HLOC * DH], F32R, kind="ExternalInput")
    wkv_d = nc.dram_tensor("wkv", [P, NKC, 2 * DH], F32R, kind="ExternalInput")
    wo_d = nc.dram_tensor("wo", [P, HLOC, D], F32R, kind="ExternalInput")
    wgt_d = nc.dram_tensor("wgt", [NH, HLOC], F32, kind="ExternalInput")
    v1s_d = nc.dram_tensor("v1s", [P, NT, DH], F32, kind="ExternalInput")
    cos_d = nc.dram_tensor("cos2", [P, NT, DH], F32, kind="ExternalInput")
    sin_d = nc.dram_tensor("sin2", [P, NT, DH], F32, kind="ExternalInput")
    msk_d = nc.dram_tensor("maskt", [P, NTB, TB], F32, kind="ExternalInput")
    out_d = nc.dram_tensor("out", [T, D], F32, kind="ExternalOutput")

    with tile.TileContext(nc) as tc:
        with (
            tc.tile_pool(name="cst", bufs=1) as cst,
            tc.tile_pool(name="res", bufs=1) as res,
        ):
            # ---- constants ----
            ident = cst.tile([P, P], F32, tag="ident")
            make_identity(nc, ident[:])
            ones_f = cst.tile([P, 1], F32, tag="ones_f")
            nc.vector.memset(ones_f[:], 1.0)
            ones_r = cst.tile([P, 1], F32R, tag="ones_r")
            nc.scalar.activation(ones_r[:], ones_f[:], AF.Copy)
            onec_f = cst.tile([1, P], F32, tag="onec_f")
            nc.vector.memset(onec_f[:], 1.0)
            onec_r = cst.tile([1, P], F32R, tag="onec_r")
            nc.scalar.activation(onec_r[:], onec_f[:], AF.Copy)
            eps_t = cst.tile([P, 1], F32, tag="eps")
            nc.vector.memset(eps_t[:], EPS)

            # ---- tensors spanning phase A -> B ----
            qt_t = res.tile([P, HLOC, T], F32R, tag="QT")     # Q^T per head
            kt_t = res.tile([P, T], F32R, tag="KT")           # K^T
            vn_t = res.tile([P, NT, DH], F32R, tag="VN")      # V natural (s-tiled)
            gt_t = [res.tile([1, T], F32, tag=f"GT{h}", name=f"GT{h}")
                    for h in range(HLOC)]

            _phase_a(nc, tc, (xt_d, wq_d, wkv_d, wgt_d, v1s_d, cos_d, sin_d,
                              qt_t, kt_t, vn_t, gt_t, ident, eps_t))

            with tc.tile_pool(name="bw", bufs=1) as bw:
                wo_t = bw.tile([P, HLOC, D], F32R, tag="wo")
                nc.sync.dma_start(wo_t[:], wo_d[:])
                yt_t = bw.tile([P, HLOC, T], F32R, tag="YT")  # y^T per head

                _phase_b(nc, tc, bw, (msk_d, qt_t, kt_t, vn_t, gt_t, yt_t,
                                      onec_r, ones_r))
                _phase_c(nc, tc, out_d, yt_t, wo_t)

    nc.compile()
    return nc


_NC_CACHE = None


def _get_nc():
    global _NC_CACHE
    if _NC_CACHE is None:
        _NC_CACHE = _build_nc()
    return _NC_CACHE


def _make_in_maps(x, pos_ids, v1, Wq, Wk, Wv, Wo, Wg, v_lamb):
    x = np.asarray(x, np.float32)
    pos_ids = np.asarray(pos_ids)
    v1 = np.asarray(v1, np.float32)
    Wq = np.asarray(Wq, np.float32)
    Wk = np.asarray(Wk, np.float32)
    Wv = np.asarray(Wv, np.float32)
    Wo = np.asarray(Wo, np.float32)
    Wg = np.asarray(Wg, np.float32)
    lamb = np.float32(np.asarray(v_lamb))

    # rope tables from pos_ids (fp32 math to match reference)
    half = DH // 2
    inv_freq = (1.0 / (np.float32(ROPE_BASE) **
                       (np.arange(half, dtype=np.float32) / np.float32(half)))
                ).astype(np.float32)
    ang = pos_ids.astype(np.float32)[:, None] * inv_freq[None, :]
    cos = np.cos(ang).astype(np.float32)
    sin = np.sin(ang).astype(np.float32)
    cos2 = _chunk_part_major(np.concatenate([cos, cos], axis=1))
    sin2 = _chunk_part_major(np.concatenate([sin, sin], axis=1))

    # mask variants [P, NTB, TB]: 0 if (c - 128*r) >= i else MASK_NEG
    i_idx = np.arange(P)[:, None, None]
    r_idx = np.arange(NTB)[None, :, None]
    c_idx = np.arange(TB)[None, None, :]
    maskt = np.where((c_idx - P * r_idx) >= i_idx, 0.0, MASK_NEG).astype(np.float32)

    xt_rounded = [
        _chunk_part_major(np.ascontiguousarray(_round_f32r(x[b]).T))
        for b in range(B)
    ]

    in_maps = []
    for c in range(8):
        b, g = divmod(c, 4)
        wq_g = _round_f32r(_chunk_part_major(Wq[:, 4 * g * DH:(4 * g + 4) * DH]))
        wkv = np.concatenate(
            [Wk[:, g * DH:(g + 1) * DH],
             (1.0 - lamb) * Wv[:, g * DH:(g + 1) * DH]], axis=1)
        wkv_g = _round_f32r(_chunk_part_major(wkv))
        wo_g = _round_f32r(_chunk_part_major(Wo[4 * g * DH:(4 * g + 4) * DH, :]))
        wgt_g = np.ascontiguousarray(Wg[4 * g:4 * g + 4, :].T)
        v1s_g = _chunk_part_major(lamb * v1[b, g])
        in_maps.append({
            "xt": xt_rounded[b],
            "wq": wq_g, "wkv": wkv_g, "wo": wo_g, "wgt": wgt_g,
            "v1s": v1s_g, "cos2": cos2, "sin2": sin2, "maskt": maskt,
        })
    return in_maps


def kernel(x, pos_ids, v1, Wq, Wk, Wv, Wo, Wg, v_lamb,
           _trace=False, _res_out=None, _tmpdir=None):
    nc = _get_nc()
    in_maps = _make_in_maps(x, pos_ids, v1, Wq, Wk, Wv, Wo, Wg, v_lamb)
    res = run_bass_kernel_spmd(nc, in_maps, list(range(8)), trace=_trace,
                               tmpdir=_tmpdir)
    if _res_out is not None:
        _res_out.append(res)
    out = np.zeros((B, T, D), np.float32)
    for c in range(8):
        b = c // 4
        out[b] += res.results[c]["out"]
    return out, np.asarray(v1, np.float32)


# revision 16
# speedup vs baseline: 24.1934x; 1.1107x over previous
"""Fused causal GQA attention block (RMSNorm+RoPE+value-residual+gated attn)
for Trainium2, SPMD over 8 NeuronCores.

Sharding: core c = b*4 + g  (b in {0,1} batch, g in {0..3} kv-head group).
Each core computes its batch's 4 q-heads (one kv head) end-to-end:
  Q/K/V projections, RMSNorm+RoPE, causal softmax (no max-sub needed:
  |scores| <= sqrt(128)), value-residual lerp (folded into Wv/v1 on host),
  sigmoid gating, and a partial output projection (its 512 rows of Wo).
Host sums the 4 partial out-projections per batch.

x is shipped pre-transposed (X^T) so no on-device transposition of the
activations is needed; all matmuls run as float32r (e8m11) on the PE at
bf16 speed. Softmax denominators are partition-reduced on the (otherwise
idle) GPSIMD engine instead of PE ones-matmuls.
"""

import os
import sys
import numpy as np

for _p in ("/opt/trn_rl_repo", "/root/.axon_site/_ro/trn_rl_repo"):
    if os.path.isdir(_p) and _p not in sys.path:
        sys.path.insert(0, _p)

import concourse.bass as bass  # noqa: E402
import concourse.mybir as mybir  # noqa: E402
import concourse.tile as tile  # noqa: E402
from concourse import bacc  # noqa: E402
from concourse.bass_utils import run_bass_kernel_spmd  # noqa: E402
from concourse.masks import make_identity  # noqa: E402

AF = mybir.ActivationFunctionType
F32 = mybir.dt.float32
F32R = mybir.dt.float32r

B, T, D = 2, 2048, 2048
NH, NKV, DH = 16, 4, 128
HLOC = NH // NKV          # 4 q heads per core
P = 128                   # partition tile
NT = T // P               # 16 token tiles
NKC = D // P              # 16 contraction chunks
NTB = 4                   # tq blocks of 512
TB = T // NTB             # 512
SCALE = float(1.0 / np.sqrt(DH))
MASK_NEG = -30000.0
EPS = float(np.finfo(np.float32).eps)
ROPE_BASE = 10000.0


def _round_f32r(x: np.ndarray) -> np.ndarray:
    """Round-to-nearest-even fp32 -> fp32r (e8m11, low 12 bits zero)."""
    x = np.ascontiguousarray(x, dtype=np.float32)
    u = x.view(np.uint32).astype(np.uint64)
    lo = u & 0xFFF
    base = u & ~np.uint64(0xFFF)
    lsb = (u >> np.uint64(12)) & 1
    rnd = (lo > 0x800) | ((lo == 0x800) & (lsb == 1))
    out = base + np.where(rnd, np.uint64(0x1000), np.uint64(0))
    return out.astype(np.uint32).view(np.float32).reshape(x.shape)


def _chunk_part_major(a: np.ndarray, chunk: int = P) -> np.ndarray:
    """[C*chunk, N] -> [chunk, C, N] with out[p, c, :] = a[c*chunk + p, :]."""
    c = a.shape[0] // chunk
    return np.ascontiguousarray(a.reshape(c, chunk, a.shape[1]).transpose(1, 0, 2))


def _phase_a(nc, tc, tensors):
    (xt_d, wq_d, wkv_d, wgt_d, v1s_d, cos_d, sin_d,
     qt_t, kt_t, vn_t, gt_t, ident, eps_t) = tensors
    HF = DH // 2
    with (
        tc.tile_pool(name="aw", bufs=1) as aw,
        tc.tile_pool(name="wka", bufs=2) as wka,
        tc.tile_pool(name="scra", bufs=1) as scra,
        tc.tile_pool(name="psa2", bufs=2, space="PSUM") as psa2,
        tc.tile_pool(name="psa1", bufs=1, space="PSUM") as psa1,
    ):
        wq_t = aw.tile([P, NKC, HLOC * DH], F32R, tag="wq")
        wkv_t = aw.tile([P, NKC, 2 * DH], F32R, tag="wkv")
        wgt_t = aw.tile([NH, HLOC], F32, tag="wgt")
        cos_t = aw.tile([P, NT, DH], F32, tag="cos")
        sin_t = aw.tile([P, NT, DH], F32, tag="sin")
        v1_t = aw.tile([P, NT, DH], F32, tag="v1")
        gt4 = aw.tile([HLOC, T], F32, tag="gt4")
        nc.sync.dma_start(wq_t[:], wq_d[:])
        nc.sync.dma_start(wkv_t[:], wkv_d[:])
        nc.sync.dma_start(wgt_t[:], wgt_d[:])
        nc.sync.dma_start(cos_t[:], cos_d[:])
        nc.sync.dma_start(sin_t[:], sin_d[:])
        nc.sync.dma_start(v1_t[:], v1s_d[:])

        for t in range(NT):
            xt_t = wka.tile([P, NKC, P], F32R, tag="xt")
            nc.sync.dma_start(xt_t[:], xt_d[:, :, t * P:(t + 1) * P])

            # gates (one tiny fp32 matmul, k=16, m=4)
            gp = psa1.tile([HLOC, P], F32, tag="gp")
            nc.tensor.matmul(
                gp[:], wgt_t[:], xt_t[0:NH, 0, :].bitcast(F32),
                start=True, stop=True,
            )
            nc.scalar.activation(gt4[:, t * P:(t + 1) * P], gp[:], AF.Sigmoid)

            # Q / KV projections
            q_ps = psa2.tile([P, HLOC * DH], F32, tag="qps")
            for kc in range(NKC):
                nc.tensor.matmul(
                    q_ps[:], xt_t[:, kc, :], wq_t[:, kc, :],
                    start=(kc == 0), stop=(kc == NKC - 1),
                )
            kv_ps = psa2.tile([P, 2 * DH], F32, tag="kvps")
            for kc in range(NKC):
                nc.tensor.matmul(
                    kv_ps[:], xt_t[:, kc, :], wkv_t[:, kc, :],
                    start=(kc == 0), stop=(kc == NKC - 1),
                )

            # V = x@Wv*(1-lamb) + lamb*v1   (Wv prescaled, v1 prescaled)
            nc.vector.tensor_add(vn_t[:, t, :], kv_ps[:, DH:2 * DH], v1_t[:, t, :])

            # rmsnorm + rope on Q (4 heads) and K
            q_sb = wka.tile([P, HLOC * DH], F32, tag="qsb")
            nc.scalar.activation(q_sb[:], q_ps[:], AF.Copy)
            k_sb = wka.tile([P, DH], F32, tag="ksb")
            nc.scalar.activation(k_sb[:], kv_ps[:, 0:DH], AF.Copy)

            ms_t = scra.tile([P, HLOC + 1], F32, tag="ms")
            sq_scr = scra.tile([P, DH], F32, tag="sqscr")
            for h in range(HLOC):
                nc.scalar.activation(
                    sq_scr[:], q_sb[:, h * DH:(h + 1) * DH], AF.Square,
                    accum_out=ms_t[:, h:h + 1],
                )
            nc.scalar.activation(sq_scr[:], k_sb[:], AF.Square,
                                 accum_out=ms_t[:, HLOC:HLOC + 1])
            rt_t = scra.tile([P, HLOC + 1], F32, tag="rt")
            nc.scalar.activation(rt_t[:], ms_t[:], AF.Sqrt,
                                 scale=float(1.0 / DH), bias=eps_t[:, 0:1])
            rs_t = scra.tile([P, HLOC + 1], F32, tag="rs")
            nc.vector.reciprocal(rs_t[:], rt_t[:])

            qc = scra.tile([P, HLOC * DH], F32, tag="qc")
            qs = scra.tile([P, HLOC * DH], F32, tag="qs")
            rot = wka.tile([P, HLOC * DH], F32, tag="rot")
            for h in range(HLOC):
                sl = slice(h * DH, (h + 1) * DH)
                nc.vector.tensor_mul(qc[:, sl], q_sb[:, sl], cos_t[:, t, :])
                nc.vector.tensor_mul(qs[:, sl], q_sb[:, sl], sin_t[:, t, :])
                nc.vector.tensor_sub(
                    rot[:, h * DH:h * DH + HF],
                    qc[:, h * DH:h * DH + HF], qs[:, h * DH + HF:(h + 1) * DH])
                nc.vector.tensor_add(
                    rot[:, h * DH + HF:(h + 1) * DH],
                    qc[:, h * DH + HF:(h + 1) * DH], qs[:, h * DH:h * DH + HF])
                nc.vector.tensor_scalar_mul(rot[:, sl], rot[:, sl], rs_t[:, h:h + 1])
                tp = psa2.tile([P, P], F32, tag="tr")
                nc.tensor.transpose(tp[:], rot[:, sl], ident[:])
                nc.scalar.activation(qt_t[:, h, t * P:(t + 1) * P], tp[:], AF.Copy)

            kc_t = scra.tile([P, DH], F32, tag="kc")
            ks_t = scra.tile([P, DH], F32, tag="ks")
            krot = wka.tile([P, DH], F32, tag="krot")
            nc.vector.tensor_mul(kc_t[:], k_sb[:], cos_t[:, t, :])
            nc.vector.tensor_mul(ks_t[:], k_sb[:], sin_t[:, t, :])
            nc.vector.tensor_sub(krot[:, 0:HF], kc_t[:, 0:HF], ks_t[:, HF:DH])
            nc.vector.tensor_add(krot[:, HF:DH], kc_t[:, HF:DH], ks_t[:, 0:HF])
            nc.vector.tensor_scalar_mul(krot[:], krot[:], rs_t[:, HLOC:HLOC + 1])
            tp = psa2.tile([P, P], F32, tag="tr")
            nc.tensor.transpose(tp[:], krot[:], ident[:])
            nc.scalar.activation(kt_t[:, t * P:(t + 1) * P], tp[:], AF.Copy)

        # extract per-head gate rows to partition-0 tiles (DMA is exempt
        # from the 32-strip partition-base rule)
        for h in range(HLOC):
            nc.sync.dma_start(gt_t[h][:], gt4[h:h + 1, :])


def _phase_bc(nc, tc, bw, tensors):
    """Attention + output projection, software-pipelined.

    Blocks ordered tb-major so the out-projection for a tq block can be
    interleaved as soon as all 4 heads' epilogues for it have fired,
    keeping the PE backlog deep (HAM stays un-throttled).
    """
    (msk_d, qt_t, kt_t, vn_t, gt_t, yt_t, onec_r, ones_r, out_d, wo_t) = tensors
    msk_t = bw.tile([P, NTB, TB], F32, tag="msk")
    nc.sync.dma_start(msk_t[:], msk_d[:])
    LOOKAHEAD = 2      # S/exp emitted this many steps ahead of y/l
    EPI_DELAY = 4      # epilogue emitted this many steps after block end
    with (
        tc.tile_pool(name="wkb", bufs=4) as wkb,
        tc.tile_pool(name="smb", bufs=2) as smb,
        tc.tile_pool(name="wkc", bufs=3) as wkc,
        tc.tile_pool(name="psb3", bufs=3, space="PSUM") as psb3,
        tc.tile_pool(name="psb2", bufs=2, space="PSUM") as psb2,
        tc.tile_pool(name="psb1", bufs=1, space="PSUM") as psb1,
    ):
        triples = []
        for tb in range(NTB):
            for h in range(HLOC):
                nj = (tb + 1) * (TB // P)
                for j in range(nj):
                    triples.append((h, tb, j, nj))
        N = len(triples)
        pt_live = {}
        blk = {}           # (h, tb) -> (y_ps, l_sb)
        epi_queue = []     # (due_step, h, tb)
        epi_done = {tb: 0 for tb in range(NTB)}
        out_emitted = set()

        def emit_s(i):
            h, tb, j, nj = triples[i]
            s_ps = psb3.tile([P, TB], F32, tag="sps")
            nc.tensor.matmul(
                s_ps[:], kt_t[:, j * P:(j + 1) * P],
                qt_t[:, h, tb * TB:(tb + 1) * TB],
                start=True, stop=True,
            )
            r = j - (tb * (TB // P))
            if r >= 0:
                nc.vector.tensor_add(s_ps[:], s_ps[:], msk_t[:, r, :])
            p_t = wkb.tile([P, TB], F32R, tag="pt")
            nc.scalar.activation(p_t[:], s_ps[:], AF.Exp, scale=SCALE)
            pt_live[i] = p_t

        def emit_yl(i):
            h, tb, j, nj = triples[i]
            if j == 0:
                y_ps = psb2.tile([P, TB], F32, tag="yps")
                l_ps = psb1.tile([1, TB], F32, tag="lps")
                l_sb = smb.tile([1, TB], F32, tag="lsb")
                blk[(h, tb)] = (y_ps, l_ps, l_sb)
            y_ps, l_ps, l_sb = blk[(h, tb)]
            p_t = pt_live.pop(i)
            nc.tensor.matmul(y_ps[:], vn_t[:, j, :], p_t[:],
                             start=(j == 0), stop=(j == nj - 1))
            nc.tensor.matmul(l_ps[:], ones_r[:], p_t[:],
                             start=(j == 0), stop=(j == nj - 1))
            if j == nj - 1:
                # move l off PSUM early (frees the bank, decouples epilogue)
                nc.scalar.activation(l_sb[:], l_ps[:], AF.Copy)

        def emit_epilogue(h, tb):
            y_ps, l_ps, l_sb = blk.pop((h, tb))
            linv = smb.tile([1, TB], F32, tag="linv")
            nc.vector.reciprocal_approx_fast(linv[:], l_sb[:])
            alpha = smb.tile([1, TB], F32R, tag="alpha")
            nc.vector.tensor_mul(alpha[:], linv[:],
                                 gt_t[h][:, tb * TB:(tb + 1) * TB])
            bc_ps = psb1.tile([P, TB], F32, tag="bcps", bufs=1)
            nc.tensor.matmul(bc_ps[:], onec_r[:], alpha[:],
                             start=True, stop=True)
            bc_sb = smb.tile([P, TB], F32, tag="bcsb")
            nc.scalar.activation(bc_sb[:], bc_ps[:], AF.Copy)
            nc.vector.tensor_mul(yt_t[:, h, tb * TB:(tb + 1) * TB],
                                 y_ps[:], bc_sb[:])
            epi_done[tb] += 1

        def emit_out(tb):
            for t in range(4 * tb, 4 * tb + 4):
                for dc in range(4):
                    o_ps = psb1.tile([P, 512], F32, tag="bcps", bufs=1)
                    for h in range(HLOC):
                        nc.tensor.matmul(
                            o_ps[:], yt_t[:, h, t * P:(t + 1) * P],
                            wo_t[:, h, dc * 512:(dc + 1) * 512],
                            start=(h == 0), stop=(h == HLOC - 1),
                        )
                    o_sb = wkc.tile([P, 512], F32, tag="osb")
                    nc.scalar.activation(o_sb[:], o_ps[:], AF.Copy)
                    nc.sync.dma_start(
                        out_d[t * P:(t + 1) * P, dc * 512:(dc + 1) * 512],
                        o_sb[:])

        for step in range(N + LOOKAHEAD):
            while epi_queue and epi_queue[0][0] <= step:
                _, eh, etb = epi_queue.pop(0)
                emit_epilogue(eh, etb)
                if epi_done[etb] == HLOC and etb not in out_emitted:
                    out_emitted.add(etb)
                    emit_out(etb)
            if step < N:
                emit_s(step)
            if step >= LOOKAHEAD:
                i = step - LOOKAHEAD
                emit_yl(i)
                h, tb, j, nj = triples[i]
                if j == nj - 1:
                    epi_queue.append((step + EPI_DELAY, h, tb))
        while epi_queue:
            _, eh, etb = epi_queue.pop(0)
            emit_epilogue(eh, etb)
            if epi_done[etb] == HLOC and etb not in out_emitted:
                out_emitted.add(etb)
                emit_out(etb)


def _build_nc():
    nc = bacc.Bacc("TRN2", target_bir_lowering=False, debug=False)

    xt_d = nc.dram_tensor("xt", [P, NKC, T], F32R, kind="ExternalInput")
    wq_d = nc.dram_tensor("wq", [P, NKC, HLOC * DH], F32R, kind="ExternalInput")
    wkv_d = nc.dram_tensor("wkv", [P, NKC, 2 * DH], F32R, kind="ExternalInput")
    wo_d = nc.dram_tensor("wo", [P, HLOC, D], F32R, kind="ExternalInput")
    wgt_d = nc.dram_tensor("wgt", [NH, HLOC], F32, kind="ExternalInput")
    v1s_d = nc.dram_tensor("v1s", [P, NT, DH], F32, kind="ExternalInput")
    cos_d = nc.dram_tensor("cos2", [P, NT, DH], F32, kind="ExternalInput")
    sin_d = nc.dram_tensor("sin2", [P, NT, DH], F32, kind="ExternalInput")
    msk_d = nc.dram_tensor("maskt", [P, NTB, TB], F32, kind="ExternalInput")
    out_d = nc.dram_tensor("out", [T, D], F32, kind="ExternalOutput")

    with tile.TileContext(nc) as tc:
        with (
            tc.tile_pool(name="cst", bufs=1) as cst,
            tc.tile_pool(name="res", bufs=1) as res,
        ):
            # ---- constants ----
            ident = cst.tile([P, P], F32, tag="ident")
            make_identity(nc, ident[:])
            ones_f = cst.tile([P, 1], F32, tag="ones_f")
            nc.vector.memset(ones_f[:], 1.0)
            ones_r = cst.tile([P, 1], F32R, tag="ones_r")
            nc.scalar.activation(ones_r[:], ones_f[:], AF.Copy)
            onec_f = cst.tile([1, P], F32, tag="onec_f")
            nc.vector.memset(onec_f[:], 1.0)
            onec_r = cst.tile([1, P], F32R, tag="onec_r")
            nc.scalar.activation(onec_r[:], onec_f[:], AF.Copy)
            eps_t = cst.tile([P, 1], F32, tag="eps")
            nc.vector.memset(eps_t[:], EPS)

            # ---- tensors spanning phase A -> B ----
            qt_t = res.tile([P, HLOC, T], F32R, tag="QT")     # Q^T per head
            kt_t = res.tile([P, T], F32R, tag="KT")           # K^T
            vn_t = res.tile([P, NT, DH], F32R, tag="VN")      # V natural (s-tiled)
            gt_t = [res.tile([1, T], F32, tag=f"GT{h}", name=f"GT{h}")
                    for h in range(HLOC)]

            _phase_a(nc, tc, (xt_d, wq_d, wkv_d, wgt_d, v1s_d, cos_d, sin_d,
                              qt_t, kt_t, vn_t, gt_t, ident, eps_t))

            with tc.tile_pool(name="bw", bufs=1) as bw:
                wo_t = bw.tile([P, HLOC, D], F32R, tag="wo")
                nc.sync.dma_start(wo_t[:], wo_d[:])
                yt_t = bw.tile([P, HLOC, T], F32R, tag="YT")  # y^T per head

                _phase_bc(nc, tc, bw, (msk_d, qt_t, kt_t, vn_t, gt_t, yt_t,
                                       onec_r, ones_r, out_d, wo_t))

    nc.compile()
    return nc


_NC_CACHE = None


def _get_nc():
    global _NC_CACHE
    if _NC_CACHE is None:
        _NC_CACHE = _build_nc()
    return _NC_CACHE


def _make_in_maps(x, pos_ids, v1, Wq, Wk, Wv, Wo, Wg, v_lamb):
    x = np.asarray(x, np.float32)
    pos_ids = np.asarray(pos_ids)
    v1 = np.asarray(v1, np.float32)
    Wq = np.asarray(Wq, np.float32)
    Wk = np.asarray(Wk, np.float32)
    Wv = np.asarray(Wv, np.float32)
    Wo = np.asarray(Wo, np.float32)
    Wg = np.asarray(Wg, np.float32)
    lamb = np.float32(np.asarray(v_lamb))

    # rope tables from pos_ids (fp32 math to match reference)
    half = DH // 2
    inv_freq = (1.0 / (np.float32(ROPE_BASE) **
                       (np.arange(half, dtype=np.float32) / np.float32(half)))
                ).astype(np.float32)
    ang = pos_ids.astype(np.float32)[:, None] * inv_freq[None, :]
    cos = np.cos(ang).astype(np.float32)
    sin = np.sin(ang).astype(np.float32)
    cos2 = _chunk_part_major(np.concatenate([cos, cos], axis=1))
    sin2 = _chunk_part_major(np.concatenate([sin, sin], axis=1))

    # mask variants [P, NTB, TB]: 0 if (c - 128*r) >= i else MASK_NEG
    i_idx = np.arange(P)[:, None, None]
    r_idx = np.arange(NTB)[None, :, None]
    c_idx = np.arange(TB)[None, None, :]
    maskt = np.where((c_idx - P * r_idx) >= i_idx, 0.0, MASK_NEG).astype(np.float32)

    xt_rounded = [
        _chunk_part_major(np.ascontiguousarray(_round_f32r(x[b]).T))
        for b in range(B)
    ]

    in_maps = []
    for c in range(8):
        b, g = divmod(c, 4)
        wq_g = _round_f32r(_chunk_part_major(Wq[:, 4 * g * DH:(4 * g + 4) * DH]))
        wkv = np.concatenate(
            [Wk[:, g * DH:(g + 1) * DH],
             (1.0 - lamb) * Wv[:, g * DH:(g + 1) * DH]], axis=1)
        wkv_g = _round_f32r(_chunk_part_major(wkv))
        wo_g = _round_f32r(_chunk_part_major(Wo[4 * g * DH:(4 * g + 4) * DH, :]))
        wgt_g = np.ascontiguousarray(Wg[4 * g:4 * g + 4, :].T)
        v1s_g = _chunk_part_major(lamb * v1[b, g])
        in_maps.append({
            "xt": xt_rounded[b],
            "wq": wq_g, "wkv": wkv_g, "wo": wo_g, "wgt": wgt_g,
            "v1s": v1s_g, "cos2": cos2, "sin2": sin2, "maskt": maskt,
        })
    return in_maps


def kernel(x, pos_ids, v1, Wq, Wk, Wv, Wo, Wg, v_lamb,
           _trace=False, _res_out=None, _tmpdir=None):
    nc = _get_nc()
    in_maps = _make_in_maps(x, pos_ids, v1, Wq, Wk, Wv, Wo, Wg, v_lamb)
    res = run_bass_kernel_spmd(nc, in_maps, list(range(8)), trace=_trace,
                               tmpdir=_tmpdir)
    if _res_out is not None:
        _res_out.append(res)
    out = np.zeros((B, T, D), np.float32)
    for c in range(8):
        b = c // 4
        out[b] += res.results[c]["out"]
    return out, np.asarray(v1, np.float32)


# revision 17
# speedup vs baseline: 26.7793x; 1.1069x over previous
"""Fused causal GQA attention block (RMSNorm+RoPE+value-residual+gated attn)
for Trainium2, SPMD over 8 NeuronCores.

Sharding: core c = b*4 + g  (b in {0,1} batch, g in {0..3} kv-head group).
Each core computes its batch's 4 q-heads (one kv head) end-to-end:
  Q/K/V projections, RMSNorm+RoPE, causal softmax (no max-sub needed:
  |scores| <= sqrt(128)), value-residual lerp (folded into Wv/v1 on host),
  sigmoid gating, and a partial output projection (its 512 rows of Wo).
Host sums the 4 partial out-projections per batch.

x is shipped pre-transposed (X^T) so no on-device transposition of the
activations is needed; all matmuls run as float32r (e8m11) on the PE at
bf16 speed. Softmax denominators are partition-reduced on the (otherwise
idle) GPSIMD engine instead of PE ones-matmuls.
"""

import os
import sys
import numpy as np

for _p in ("/opt/trn_rl_repo", "/root/.axon_site/_ro/trn_rl_repo"):
    if os.path.isdir(_p) and _p not in sys.path:
        sys.path.insert(0, _p)

import concourse.bass as bass  # noqa: E402
import concourse.mybir as mybir  # noqa: E402
import concourse.tile as tile  # noqa: E402
from concourse import bacc  # noqa: E402
from concourse.bass_utils import run_bass_kernel_spmd  # noqa: E402
from concourse.masks import make_identity  # noqa: E402

AF = mybir.ActivationFunctionType
F32 = mybir.dt.float32
F32R = mybir.dt.float32r

B, T, D = 2, 2048, 2048
NH, NKV, DH = 16, 4, 128
HLOC = NH // NKV          # 4 q heads per core
P = 128                   # partition tile
NT = T // P               # 16 token tiles
NKC = D // P              # 16 contraction chunks
NTB = 4                   # tq blocks of 512
TB = T // NTB             # 512
SCALE = float(1.0 / np.sqrt(DH))
MASK_NEG = -30000.0
EPS = float(np.finfo(np.float32).eps)
ROPE_BASE = 10000.0


def _round_f32r(x: np.ndarray) -> np.ndarray:
    """Round-to-nearest-even fp32 -> fp32r (e8m11, low 12 bits zero)."""
    x = np.ascontiguousarray(x, dtype=np.float32)
    u = x.view(np.uint32).astype(np.uint64)
    lo = u & 0xFFF
    base = u & ~np.uint64(0xFFF)
    lsb = (u >> np.uint64(12)) & 1
    rnd = (lo > 0x800) | ((lo == 0x800) & (lsb == 1))
    out = base + np.where(rnd, np.uint64(0x1000), np.uint64(0))
    return out.astype(np.uint32).view(np.float32).reshape(x.shape)


def _chunk_part_major(a: np.ndarray, chunk: int = P) -> np.ndarray:
    """[C*chunk, N] -> [chunk, C, N] with out[p, c, :] = a[c*chunk + p, :]."""
    c = a.shape[0] // chunk
    return np.ascontiguousarray(a.reshape(c, chunk, a.shape[1]).transpose(1, 0, 2))


def _phase_a(nc, tc, tensors):
    (xt_d, wq_d, wkv_d, wgt_d, v1s_d, cos_d, sin_d,
     qt_t, kt_t, vn_t, gt_t, ident, eps_t) = tensors
    HF = DH // 2
    with (
        tc.tile_pool(name="aw", bufs=1) as aw,
        tc.tile_pool(name="wka", bufs=2) as wka,
        tc.tile_pool(name="scra", bufs=1) as scra,
        tc.tile_pool(name="psa2", bufs=2, space="PSUM") as psa2,
        tc.tile_pool(name="psa1", bufs=1, space="PSUM") as psa1,
    ):
        wq_t = aw.tile([P, NKC, HLOC * DH], F32R, tag="wq")
        wkv_t = aw.tile([P, NKC, 2 * DH], F32R, tag="wkv")
        wgt_t = aw.tile([NH, HLOC], F32, tag="wgt")
        cos_t = aw.tile([P, NT, DH], F32, tag="cos")
        sin_t = aw.tile([P, NT, DH], F32, tag="sin")
        v1_t = aw.tile([P, NT, DH], F32, tag="v1")
        gt4 = aw.tile([HLOC, T], F32, tag="gt4")
        nc.sync.dma_start(wq_t[:], wq_d[:])
        nc.sync.dma_start(wkv_t[:], wkv_d[:])
        nc.sync.dma_start(wgt_t[:], wgt_d[:])
        nc.sync.dma_start(cos_t[:], cos_d[:])
        nc.sync.dma_start(sin_t[:], sin_d[:])
        nc.sync.dma_start(v1_t[:], v1s_d[:])

        for t in range(NT):
            xt_t = wka.tile([P, NKC, P], F32R, tag="xt")
            nc.sync.dma_start(xt_t[:], xt_d[:, :, t * P:(t + 1) * P])

            # gates (one tiny fp32 matmul, k=16, m=4)
            gp = psa1.tile([HLOC, P], F32, tag="gp")
            nc.tensor.matmul(
                gp[:], wgt_t[:], xt_t[0:NH, 0, :].bitcast(F32),
                start=True, stop=True,
            )
            nc.scalar.activation(gt4[:, t * P:(t + 1) * P], gp[:], AF.Sigmoid)

            # Q / KV projections
            q_ps = psa2.tile([P, HLOC * DH], F32, tag="qps")
            for kc in range(NKC):
                nc.tensor.matmul(
                    q_ps[:], xt_t[:, kc, :], wq_t[:, kc, :],
                    start=(kc == 0), stop=(kc == NKC - 1),
                )
            kv_ps = psa2.tile([P, 2 * DH], F32, tag="kvps")
            for kc in range(NKC):
                nc.tensor.matmul(
                    kv_ps[:], xt_t[:, kc, :], wkv_t[:, kc, :],
                    start=(kc == 0), stop=(kc == NKC - 1),
                )

            # V = x@Wv*(1-lamb) + lamb*v1   (Wv prescaled, v1 prescaled)
            nc.vector.tensor_add(vn_t[:, t, :], kv_ps[:, DH:2 * DH], v1_t[:, t, :])

            # rmsnorm + rope on Q (4 heads) and K
            q_sb = wka.tile([P, HLOC * DH], F32, tag="qsb")
            nc.scalar.activation(q_sb[:], q_ps[:], AF.Copy)
            k_sb = wka.tile([P, DH], F32, tag="ksb")
            nc.scalar.activation(k_sb[:], kv_ps[:, 0:DH], AF.Copy)

            ms_t = scra.tile([P, HLOC + 1], F32, tag="ms")
            sq_scr = scra.tile([P, DH], F32, tag="sqscr")
            for h in range(HLOC):
                nc.scalar.activation(
                    sq_scr[:], q_sb[:, h * DH:(h + 1) * DH], AF.Square,
                    accum_out=ms_t[:, h:h + 1],
                )
            nc.scalar.activation(sq_scr[:], k_sb[:], AF.Square,
                                 accum_out=ms_t[:, HLOC:HLOC + 1])
            rt_t = scra.tile([P, HLOC + 1], F32, tag="rt")
            nc.scalar.activation(rt_t[:], ms_t[:], AF.Sqrt,
                                 scale=float(1.0 / DH), bias=eps_t[:, 0:1])
            rs_t = scra.tile([P, HLOC + 1], F32, tag="rs")
            nc.vector.reciprocal(rs_t[:], rt_t[:])

            qc = scra.tile([P, HLOC * DH], F32, tag="qc")
            qs = scra.tile([P, HLOC * DH], F32, tag="qs")
            rot = wka.tile([P, HLOC * DH], F32, tag="rot")
            for h in range(HLOC):
                sl = slice(h * DH, (h + 1) * DH)
                nc.vector.tensor_mul(qc[:, sl], q_sb[:, sl], cos_t[:, t, :])
                nc.vector.tensor_mul(qs[:, sl], q_sb[:, sl], sin_t[:, t, :])
                nc.vector.tensor_sub(
                    rot[:, h * DH:h * DH + HF],
                    qc[:, h * DH:h * DH + HF], qs[:, h * DH + HF:(h + 1) * DH])
                nc.vector.tensor_add(
                    rot[:, h * DH + HF:(h + 1) * DH],
                    qc[:, h * DH + HF:(h + 1) * DH], qs[:, h * DH:h * DH + HF])
                nc.vector.tensor_scalar_mul(rot[:, sl], rot[:, sl], rs_t[:, h:h + 1])
                tp = psa2.tile([P, P], F32, tag="tr")
                nc.tensor.transpose(tp[:], rot[:, sl], ident[:])
                nc.scalar.activation(qt_t[:, h, t * P:(t + 1) * P], tp[:], AF.Copy)

            kc_t = scra.tile([P, DH], F32, tag="kc")
            ks_t = scra.tile([P, DH], F32, tag="ks")
            krot = wka.tile([P, DH], F32, tag="krot")
            nc.vector.tensor_mul(kc_t[:], k_sb[:], cos_t[:, t, :])
            nc.vector.tensor_mul(ks_t[:], k_sb[:], sin_t[:, t, :])
            nc.vector.tensor_sub(krot[:, 0:HF], kc_t[:, 0:HF], ks_t[:, HF:DH])
            nc.vector.tensor_add(krot[:, HF:DH], kc_t[:, HF:DH], ks_t[:, 0:HF])
            nc.vector.tensor_scalar_mul(krot[:], krot[:], rs_t[:, HLOC:HLOC + 1])
            tp = psa2.tile([P, P], F32, tag="tr")
            nc.tensor.transpose(tp[:], krot[:], ident[:])
            nc.scalar.activation(kt_t[:, t * P:(t + 1) * P], tp[:], AF.Copy)

        # extract per-head gate rows to partition-0 tiles (DMA is exempt
        # from the 32-strip partition-base rule)
        for h in range(HLOC):
            nc.sync.dma_start(gt_t[h][:], gt4[h:h + 1, :])


def _phase_bc(nc, tc, bw, tensors):
    """Attention + output projection, software-pipelined.

    Blocks ordered tb-major so the out-projection for a tq block can be
    interleaved as soon as all 4 heads' epilogues for it have fired,
    keeping the PE backlog deep (HAM stays un-throttled).
    """
    (msk_t, qt_t, kt_t, vn_t, gt_t, yt_t, onec_r, ones_r, out_d, wo_t) = tensors
    LOOKAHEAD = 2      # S/exp emitted this many steps ahead of y/l
    EPI_DELAY = 4      # epilogue emitted this many steps after block end
    with (
        tc.tile_pool(name="wkb", bufs=4) as wkb,
        tc.tile_pool(name="smb", bufs=2) as smb,
        tc.tile_pool(name="wkc", bufs=3) as wkc,
        tc.tile_pool(name="psb3", bufs=3, space="PSUM") as psb3,
        tc.tile_pool(name="psb2", bufs=2, space="PSUM") as psb2,
        tc.tile_pool(name="psb1", bufs=1, space="PSUM") as psb1,
    ):
        triples = []
        for tb in range(NTB):
            for h in range(HLOC):
                nj = (tb + 1) * (TB // P)
                for j in range(nj):
                    triples.append((h, tb, j, nj))
        N = len(triples)
        pt_live = {}
        blk = {}           # (h, tb) -> (y_ps, l_sb)
        epi_queue = []     # (due_step, h, tb)
        epi_done = {tb: 0 for tb in range(NTB)}
        out_emitted = set()

        def emit_s(i):
            h, tb, j, nj = triples[i]
            s_ps = psb3.tile([P, TB], F32, tag="sps")
            nc.tensor.matmul(
                s_ps[:], kt_t[:, j * P:(j + 1) * P],
                qt_t[:, h, tb * TB:(tb + 1) * TB],
                start=True, stop=True,
            )
            r = j - (tb * (TB // P))
            if r >= 0:
                nc.vector.tensor_add(s_ps[:], s_ps[:], msk_t[:, r, :])
            p_t = wkb.tile([P, TB], F32R, tag="pt")
            nc.scalar.activation(p_t[:], s_ps[:], AF.Exp, scale=SCALE)
            pt_live[i] = p_t

        def emit_yl(i):
            h, tb, j, nj = triples[i]
            if j == 0:
                y_ps = psb2.tile([P, TB], F32, tag="yps")
                l_ps = psb1.tile([1, TB], F32, tag="lps")
                l_sb = smb.tile([1, TB], F32, tag="lsb")
                blk[(h, tb)] = (y_ps, l_ps, l_sb)
            y_ps, l_ps, l_sb = blk[(h, tb)]
            p_t = pt_live.pop(i)
            nc.tensor.matmul(y_ps[:], vn_t[:, j, :], p_t[:],
                             start=(j == 0), stop=(j == nj - 1))
            nc.tensor.matmul(l_ps[:], ones_r[:], p_t[:],
                             start=(j == 0), stop=(j == nj - 1))
            if j == nj - 1:
                # move l off PSUM early (frees the bank, decouples epilogue)
                nc.scalar.activation(l_sb[:], l_ps[:], AF.Copy)

        def emit_epilogue(h, tb):
            y_ps, l_ps, l_sb = blk.pop((h, tb))
            linv = smb.tile([1, TB], F32, tag="linv")
            nc.vector.reciprocal_approx_fast(linv[:], l_sb[:])
            alpha = smb.tile([1, TB], F32R, tag="alpha")
            nc.vector.tensor_mul(alpha[:], linv[:],
                                 gt_t[h][:, tb * TB:(tb + 1) * TB])
            bc_ps = psb1.tile([P, TB], F32, tag="bcps", bufs=2)
            nc.tensor.matmul(bc_ps[:], onec_r[:], alpha[:],
                             start=True, stop=True)
            bc_sb = smb.tile([P, TB], F32, tag="bcsb")
            nc.scalar.activation(bc_sb[:], bc_ps[:], AF.Copy)
            nc.vector.tensor_mul(yt_t[:, h, tb * TB:(tb + 1) * TB],
                                 y_ps[:], bc_sb[:])
            epi_done[tb] += 1

        def emit_out(tb):
            for t in range(4 * tb, 4 * tb + 4):
                for dc in range(4):
                    o_ps = psb1.tile([P, 512], F32, tag="bcps", bufs=2)
                    for h in range(HLOC):
                        nc.tensor.matmul(
                            o_ps[:], yt_t[:, h, t * P:(t + 1) * P],
                            wo_t[:, h, dc * 512:(dc + 1) * 512],
                            start=(h == 0), stop=(h == HLOC - 1),
                        )
                    o_sb = wkc.tile([P, 512], F32, tag="osb")
                    nc.scalar.activation(o_sb[:], o_ps[:], AF.Copy)
                    nc.sync.dma_start(
                        out_d[t * P:(t + 1) * P, dc * 512:(dc + 1) * 512],
                        o_sb[:])

        for step in range(N + LOOKAHEAD):
            while epi_queue and epi_queue[0][0] <= step:
                _, eh, etb = epi_queue.pop(0)
                emit_epilogue(eh, etb)
                if epi_done[etb] == HLOC and etb not in out_emitted:
                    out_emitted.add(etb)
                    emit_out(etb)
            if step < N:
                emit_s(step)
            if step >= LOOKAHEAD:
                i = step - LOOKAHEAD
                emit_yl(i)
                h, tb, j, nj = triples[i]
                if j == nj - 1:
                    epi_queue.append((step + EPI_DELAY, h, tb))
        while epi_queue:
            _, eh, etb = epi_queue.pop(0)
            emit_epilogue(eh, etb)
            if epi_done[etb] == HLOC and etb not in out_emitted:
                out_emitted.add(etb)
                emit_out(etb)


def _build_nc():
    nc = bacc.Bacc("TRN2", target_bir_lowering=False, debug=False)

    xt_d = nc.dram_tensor("xt", [P, NKC, T], F32R, kind="ExternalInput")
    wq_d = nc.dram_tensor("wq", [P, NKC, HLOC * DH], F32R, kind="ExternalInput")
    wkv_d = nc.dram_tensor("wkv", [P, NKC, 2 * DH], F32R, kind="ExternalInput")
    wo_d = nc.dram_tensor("wo", [P, HLOC, D], F32R, kind="ExternalInput")
    wgt_d = nc.dram_tensor("wgt", [NH, HLOC], F32, kind="ExternalInput")
    v1s_d = nc.dram_tensor("v1s", [P, NT, DH], F32, kind="ExternalInput")
    cos_d = nc.dram_tensor("cos2", [P, NT, DH], F32, kind="ExternalInput")
    sin_d = nc.dram_tensor("sin2", [P, NT, DH], F32, kind="ExternalInput")
    msk_d = nc.dram_tensor("maskt", [P, NTB, TB], F32, kind="ExternalInput")
    out_d = nc.dram_tensor("out", [T, D], F32, kind="ExternalOutput")

    with tile.TileContext(nc) as tc:
        with (
            tc.tile_pool(name="cst", bufs=1) as cst,
            tc.tile_pool(name="res", bufs=1) as res,
        ):
            # ---- constants ----
            ident = cst.tile([P, P], F32, tag="ident")
            make_identity(nc, ident[:])
            ones_f = cst.tile([P, 1], F32, tag="ones_f")
            nc.vector.memset(ones_f[:], 1.0)
            ones_r = cst.tile([P, 1], F32R, tag="ones_r")
            nc.scalar.activation(ones_r[:], ones_f[:], AF.Copy)
            onec_f = cst.tile([1, P], F32, tag="onec_f")
            nc.vector.memset(onec_f[:], 1.0)
            onec_r = cst.tile([1, P], F32R, tag="onec_r")
            nc.scalar.activation(onec_r[:], onec_f[:], AF.Copy)
            eps_t = cst.tile([P, 1], F32, tag="eps")
            nc.vector.memset(eps_t[:], EPS)

            # ---- tensors spanning phase A -> B ----
            qt_t = res.tile([P, HLOC, T], F32R, tag="QT")     # Q^T per head
            kt_t = res.tile([P, T], F32R, tag="KT")           # K^T
            vn_t = res.tile([P, NT, DH], F32R, tag="VN")      # V natural (s-tiled)
            gt_t = [res.tile([1, T], F32, tag=f"GT{h}", name=f"GT{h}")
                    for h in range(HLOC)]
            msk_t = res.tile([P, NTB, TB], F32, tag="msk")
            nc.sync.dma_start(msk_t[:], msk_d[:])

            _phase_a(nc, tc, (xt_d, wq_d, wkv_d, wgt_d, v1s_d, cos_d, sin_d,
                              qt_t, kt_t, vn_t, gt_t, ident, eps_t))

            with tc.tile_pool(name="bw", bufs=1) as bw:
                wo_t = bw.tile([P, HLOC, D], F32R, tag="wo")
                for h in range(HLOC):
                    nc.sync.dma_start(wo_t[:, h, :], wo_d[:, h, :])
                yt_t = bw.tile([P, HLOC, T], F32R, tag="YT")  # y^T per head

                _phase_bc(nc, tc, bw, (msk_t, qt_t, kt_t, vn_t, gt_t, yt_t,
                                       onec_r, ones_r, out_d, wo_t))

    nc.compile()
    return nc


_NC_CACHE = None


def _get_nc():
    global _NC_CACHE
    if _NC_CACHE is None:
        _NC_CACHE = _build_nc()
    return _NC_CACHE


def _make_in_maps(x, pos_ids, v1, Wq, Wk, Wv, Wo, Wg, v_lamb):
    x = np.asarray(x, np.float32)
    pos_ids = np.asarray(pos_ids)
    v1 = np.asarray(v1, np.float32)
    Wq = np.asarray(Wq, np.float32)
    Wk = np.asarray(Wk, np.float32)
    Wv = np.asarray(Wv, np.float32)
    Wo = np.asarray(Wo, np.float32)
    Wg = np.asarray(Wg, np.float32)
    lamb = np.float32(np.asarray(v_lamb))

    # rope tables from pos_ids (fp32 math to match reference)
    half = DH // 2
    inv_freq = (1.0 / (np.float32(ROPE_BASE) **
                       (np.arange(half, dtype=np.float32) / np.float32(half)))
                ).astype(np.float32)
    ang = pos_ids.astype(np.float32)[:, None] * inv_freq[None, :]
    cos = np.cos(ang).astype(np.float32)
    sin = np.sin(ang).astype(np.float32)
    cos2 = _chunk_part_major(np.concatenate([cos, cos], axis=1))
    sin2 = _chunk_part_major(np.concatenate([sin, sin], axis=1))

    # mask variants [P, NTB, TB]: 0 if (c - 128*r) >= i else MASK_NEG
    i_idx = np.arange(P)[:, None, None]
    r_idx = np.arange(NTB)[None, :, None]
    c_idx = np.arange(TB)[None, None, :]
    maskt = np.where((c_idx - P * r_idx) >= i_idx, 0.0, MASK_NEG).astype(np.float32)

    xt_rounded = [
        _chunk_part_major(np.ascontiguousarray(_round_f32r(x[b]).T))
        for b in range(B)
    ]

    in_maps = []
    for c in range(8):
        b, g = divmod(c, 4)
        wq_g = _round_f32r(_chunk_part_major(Wq[:, 4 * g * DH:(4 * g + 4) * DH]))
        wkv = np.concatenate(
            [Wk[:, g * DH:(g + 1) * DH],
             (1.0 - lamb) * Wv[:, g * DH:(g + 1) * DH]], axis=1)
        wkv_g = _round_f32r(_chunk_part_major(wkv))
        wo_g = _round_f32r(_chunk_part_major(Wo[4 * g * DH:(4 * g + 4) * DH, :]))
        wgt_g = np.ascontiguousarray(Wg[4 * g:4 * g + 4, :].T)
        v1s_g = _chunk_part_major(lamb * v1[b, g])
        in_maps.append({
            "xt": xt_rounded[b],
            "wq": wq_g, "wkv": wkv_g, "wo": wo_g, "wgt": wgt_g,
            "v1s": v1s_g, "cos2": cos2, "sin2": sin2, "maskt": maskt,
        })
    return in_maps


def kernel(x, pos_ids, v1, Wq, Wk, Wv, Wo, Wg, v_lamb,
           _trace=False, _res_out=None, _tmpdir=None):
    nc = _get_nc()
    in_maps = _make_in_maps(x, pos_ids, v1, Wq, Wk, Wv, Wo, Wg, v_lamb)
    res = run_bass_kernel_spmd(nc, in_maps, list(range(8)), trace=_trace,
                               tmpdir=_tmpdir)
    if _res_out is not None:
        _res_out.append(res)
    out = np.zeros((B, T, D), np.float32)
    for c in range(8):
        b = c // 4
        out[b] += res.results[c]["out"]
    return out, np.asarray(v1, np.float32)
